# revision 1
# baseline (speedup 1.0000x reference)
"""Trainium2 Bass kernel: cosine-attention + positional-adjacency mix + BiLSTM + softmax classifier.

Model (per sample, reference semantics):
    Xn   = X / ||X||_row
    Xa   = (Xn Xn^T) @ A_D @ X          (A_D = row-normalized exp(-|i-j|/8), constant)
    h    = BiLSTM(Xa)                    (fwd + bwd, H=256)
    out  = softmax(h @ Wc + bc)

Strategy: data-parallel over batch (4 samples / core x 8 cores). All device
matmuls in bf16 with fp32 PSUM accumulation. The whole feedforward runs in
"transposed" layout so the LSTM gate math operates on 128-partition tiles:
    Xa^T = X^T @ (A_D^T @ (Xn Xn^T))     -- lhsT operands are natural-layout
    gx^T = Wx^T @ Xa^T  (+b)             -- [4H, T] per direction
LSTM recurrence keeps z^T tiles [128, 8*B]; gates are host-permuted to
[i, f, o, g] so ACT does one Sigmoid (i,f,o) + one Tanh (g) per step.
h is stored directly as bf16 in a (T+1)-slot ring ("hstore") whose slices are
the matmul moving operands of the next step -- no per-step transposes/copies.

Wall-clock-per-call optimizations (the metric is dominated by host<->device
transfer + per-call program-size-proportional overhead, NOT device execute
time, which measures as negligible):
  - X is shipped as globally-scaled int8 (8 MB instead of 32 MB f32). The
    scale cancels exactly in the cosine normalization and is folded into Wx
    on the host for the value path, so the device never sees it.
  - Weights are NOT replicated 8x: each core uploads a distinct 1/8 column
    shard of the packed weights (one "PK" blob per core, with the f32
    bias sections embedded via bitcast) and an on-device AllGather
    reconstructs the full set on every core.
  - The positional adjacency A_D is generated on device (iota/abs/exp).
  - The BiLSTM recurrence and the per-sample feedforward run inside tc.For_i
    hardware loops with ds() dynamic offsets: ~1k static instructions
    instead of ~24k, which cuts ~1s/call of per-call NEFF handling overhead.
  - Output is written as bf16 (3.2 MB) and widened to f32 on host.
  - The jax persistent compilation cache is enabled so repeat calls skip the
    XLA/neuronx recompile.
"""

import os
import numpy as np
import ml_dtypes

os.environ.setdefault("JAX_COMPILATION_CACHE_DIR", "/tmp/jaxcache")
try:
    import jax
    jax.config.update("jax_compilation_cache_dir",
                      os.environ["JAX_COMPILATION_CACHE_DIR"])
    jax.config.update("jax_persistent_cache_min_entry_size_bytes", -1)
    jax.config.update("jax_persistent_cache_min_compile_time_secs", 0)
except Exception:
    pass

import concourse.bass as bass
from concourse.bass import ds
import concourse.mybir as mybir
import concourse.bacc as bacc
import concourse.tile as tile
from concourse import bass_utils

F32 = mybir.dt.float32
BF16 = mybir.dt.bfloat16
AF = mybir.ActivationFunctionType
ALU = mybir.AluOpType
BF16NP = ml_dtypes.bfloat16

B_ALL, T_FULL, D, H, V = 32, 512, 512, 256, 96
SIGMA = 8.0
NCORES = 8
BL = B_ALL // NCORES          # samples per core
G4 = 4 * H                    # 1024 gate dims (permuted order i,f,o,g)
NM = G4 // 128                # 8 gate m-tiles
KD = D // 128                 # 4
KH = H // 128                 # 2

# packed-weight shard layout (columns per core): WXf | WXb | WHf | WHb | WCp
WXS = KD * G4 // NCORES       # 512
WHS = KH * G4 // NCORES       # 256
VP = 392                      # WC cols padded 388 -> 392 (divisible by 8)
WCS = VP // NCORES            # 49
SH = 2 * WXS + 2 * WHS + WCS  # 1585 shard cols
SHP = SH + 1                  # pad col so f32 sections sit at even offsets
PKC = SHP + 2 * (2 * NM) + 2 * (V + 1)  # + f32 BIAS/BCREP as raw bf16
USE_ALLGATHER = True
# Ship X as globally-scaled int8 (8 MB instead of 16 MB bf16). The global
# scale cancels exactly in the cosine normalization; for the value path it is
# folded into Wx on the host, so the device never sees it.
X_INT8 = True
I8 = mybir.dt.int8


def _host_stationary(q):
    """[R, C] -> [128, (R//128)*C]; k-th col-block = rows k*128:(k+1)*128."""
    r, c = q.shape
    return np.ascontiguousarray(
        q.reshape(r // 128, 128, c).transpose(1, 0, 2).reshape(128, (r // 128) * c)
    )


def _permute_gates(w):
    """Reorder last-dim gate blocks [i,f,g,o] -> [i,f,o,g]."""
    i, f, g, o = np.split(w, 4, axis=-1)
    return np.concatenate([i, f, o, g], axis=-1)


def build_program(t_param=T_FULL, n_devices=NCORES, bl=BL, reps=1,
                  use_allgather=USE_ALLGATHER, x_int8=X_INT8):
    T = t_param
    KT = T // 128
    nc = bacc.Bacc(
        "TRN2", target_bir_lowering=False, debug=False, enable_asserts=False,
        num_devices=n_devices,
    )

    x_in = nc.dram_tensor("XBF", [bl, T, D], I8 if x_int8 else BF16,
                          kind="ExternalInput")
    out_d = nc.dram_tensor("OUT", [bl, T, V + 1], BF16, kind="ExternalOutput")

    if use_allgather:
        pk_in = nc.dram_tensor("PK", [128, PKC], BF16, kind="ExternalInput")
        bias_in = pk_in[:, SHP:SHP + 4 * NM].bitcast(F32)
        bc_in = pk_in[:, SHP + 4 * NM:PKC].bitcast(F32)
        wbnc = nc.dram_tensor("WBNC", [128, SH], BF16)
        wall = nc.dram_tensor("WALL", [n_devices, 128, SH], BF16,
                              addr_space="Shared")
    else:
        bias_in = nc.dram_tensor("BIAS", [128, 2 * NM], F32,
                                 kind="ExternalInput")[:]
        bc_in = nc.dram_tensor("BCREP", [128, V + 1], F32,
                               kind="ExternalInput")[:]
        wx_in = {d: nc.dram_tensor(f"WX{d}", [128, KD * G4], BF16,
                                   kind="ExternalInput") for d in "fb"}
        wh_in = {d: nc.dram_tensor(f"WH{d}", [128, KH * G4], BF16,
                                   kind="ExternalInput") for d in "fb"}
        wc_in = nc.dram_tensor("WC", [128, VP], BF16, kind="ExternalInput")

    B8 = 2 * bl    # h-cols per hstore slot
    GB = NM * bl   # z free cols (8*B)

    from contextlib import ExitStack
    with tile.TileContext(nc) as tc:
        _rep = ExitStack()
        if reps > 1:
            _rep.enter_context(tc.For_i(0, reps, 1))
        with (
            tc.tile_pool(name="const", bufs=1) as cpool,
            tc.tile_pool(name="gates", bufs=1) as gpool,
            tc.tile_pool(name="state", bufs=1) as spool,
        ):
            if use_allgather:
                nc.sync.dma_start(wbnc[:], pk_in[:, :SH])
                nc.gpsimd.collective_compute(
                    "AllGather", ALU.bypass,
                    replica_groups=[list(range(n_devices))],
                    ins=[wbnc[:].opt()], outs=[wall[:].opt()])
                c0 = 0
                wsrc = {}
                for nm_, w_ in [("wxf", WXS), ("wxb", WXS), ("whf", WHS),
                                ("whb", WHS), ("wc", WCS)]:
                    wsrc[nm_] = wall[:, :, c0:c0 + w_].rearrange("r p c -> p r c")
                    c0 += w_
            wx_sb = {}
            wh_sb = {}
            for d in "fb":
                wx_sb[d] = cpool.tile([128, KD * G4], BF16, name=f"wx_{d}")
                nc.sync.dma_start(
                    wx_sb[d][:], wsrc[f"wx{d}"] if use_allgather else wx_in[d][:])
                wh_sb[d] = cpool.tile([128, KH * G4], BF16, name=f"wh_{d}")
                nc.sync.dma_start(
                    wh_sb[d][:], wsrc[f"wh{d}"] if use_allgather else wh_in[d][:])
            wc_sb = cpool.tile([128, VP], BF16)
            nc.sync.dma_start(wc_sb[:], wsrc["wc"] if use_allgather else wc_in[:])
            bias_sb = cpool.tile([128, 2 * NM], F32)
            nc.sync.dma_start(bias_sb[:], bias_in)
            bc_sb = cpool.tile([128, V + 1], F32)
            nc.sync.dma_start(bc_sb[:], bc_in)

            # A_D generated on device: ad_sb[p, k*T+c] = AD[k*128+p, c]
            ad_sb = cpool.tile([128, KT * T], BF16)
            with tc.tile_pool(name="adgen", bufs=2) as agp:
                for k in range(KT):
                    df = agp.tile([128, T], F32, tag="df")
                    nc.gpsimd.iota(df[:], pattern=[[1, T]], base=-(k * 128),
                                   channel_multiplier=-1,
                                   allow_small_or_imprecise_dtypes=True)
                    ab = agp.tile([128, T], F32, tag="ab")
                    nc.scalar.activation(ab[:], df[:], AF.Abs)
                    ex = agp.tile([128, T], F32, tag="ex")
                    ssum = agp.tile([128, 1], F32, tag="ssum")
                    nc.scalar.activation(ex[:], ab[:], AF.Exp,
                                         scale=-1.0 / SIGMA, accum_out=ssum[:])
                    rs = agp.tile([128, 1], F32, tag="rs")
                    nc.vector.reciprocal(rs[:], ssum[:])
                    nc.vector.tensor_scalar_mul(
                        ad_sb[:, k * T:(k + 1) * T], ex[:], rs[:])

            gates = {d: gpool.tile([128, NM * bl * T], BF16, name=f"gates_{d}")
                     for d in "fb"}
            hstore = {d: spool.tile([128, (T + 1) * B8], BF16, name=f"hstore_{d}")
                      for d in "fb"}
            cstate = {d: spool.tile([128, B8], F32, name=f"cstate_{d}") for d in "fb"}
            for d in "fb":
                z0 = 0 if d == "f" else T
                nc.vector.memset(hstore[d][:, z0 * B8:(z0 + 1) * B8], 0.0)
                nc.vector.memset(cstate[d][:], 0.0)

            # ---------------- Phase A: feedforward per sample ----------------
            with (
                tc.tile_pool(name="xb", bufs=2) as xbp,
                tc.tile_pool(name="mats", bufs=2) as mpool,
                tc.tile_pool(name="small", bufs=4) as smpool,
                tc.tile_pool(name="ps", bufs=4, space="PSUM") as pspool,
            ):
                with tc.For_i(0, bl, 1) as s:
                    x_bf = xbp.tile([128, KT, D], BF16, tag="x_bf")
                    if x_int8:
                        x8 = xbp.tile([128, KT, D], I8, tag="x8")
                        nc.sync.dma_start(
                            x8[:].rearrange("p (o k) d -> p o k d", o=1),
                            x_in[ds(s, 1)].rearrange("o (k p) d -> p o k d", p=128))
                        nc.vector.tensor_copy(x_bf[:], x8[:])
                    else:
                        nc.sync.dma_start(
                            x_bf[:].rearrange("p (o k) d -> p o k d", o=1),
                            x_in[ds(s, 1)].rearrange("o (k p) d -> p o k d", p=128))
                    xn_bf = xbp.tile([128, KT, D], BF16, tag="xn_bf")
                    xnt_bf = xbp.tile([128, KD, T], BF16, tag="xnt_bf")
                    dump = smpool.tile([128, D], BF16, tag="dump")
                    for k in range(KT):
                        ss = smpool.tile([128, 1], F32, tag="ss")
                        nc.scalar.activation(dump[:], x_bf[:, k, :], AF.Square,
                                             accum_out=ss[:])
                        sn = smpool.tile([128, 1], F32, tag="sn")
                        nc.scalar.activation(sn[:], ss[:], AF.Sqrt)
                        rn = smpool.tile([128, 1], F32, tag="rn")
                        nc.vector.reciprocal(rn[:], sn[:])
                        nc.vector.tensor_scalar_mul(xn_bf[:, k, :], x_bf[:, k, :], rn[:])
                    # Xn^T via DMA block transposes
                    for ti in range(KT):
                        for dj in range(KD):
                            nc.sync.dma_start_transpose(
                                xnt_bf[:, dj, ti * 128:(ti + 1) * 128],
                                xn_bf[:, ti, dj * 128:(dj + 1) * 128])
                    # A_S = Xn Xn^T   [T, T]
                    as_bf = mpool.tile([128, KT, T], BF16, tag="as_bf")
                    for m in range(KT):
                        ps = pspool.tile([128, T], F32, tag="psA")
                        for k in range(KD):
                            nc.tensor.matmul(
                                ps[:], xnt_bf[:, k, m * 128:(m + 1) * 128],
                                xnt_bf[:, k, :], start=(k == 0), stop=(k == KD - 1))
                        nc.vector.tensor_copy(as_bf[:, m, :], ps[:])
                    # P = A_D^T @ A_S
                    p_bf = mpool.tile([128, KT, T], BF16, tag="p_bf")
                    for m in range(KT):
                        ps = pspool.tile([128, T], F32, tag="psA")
                        for k in range(KT):
                            nc.tensor.matmul(
                                ps[:], ad_sb[:, k * T + m * 128:k * T + (m + 1) * 128],
                                as_bf[:, k, :], start=(k == 0), stop=(k == KT - 1))
                        nc.vector.tensor_copy(p_bf[:, m, :], ps[:])
                    # Xa^T = X^T @ P   [D, T]
                    xat_bf = mpool.tile([128, KD, T], BF16, tag="xat_bf")
                    for m in range(KD):
                        ps = pspool.tile([128, T], F32, tag="psA")
                        for k in range(KT):
                            nc.tensor.matmul(
                                ps[:], x_bf[:, k, m * 128:(m + 1) * 128],
                                p_bf[:, k, :], start=(k == 0), stop=(k == KT - 1))
                        nc.vector.tensor_copy(xat_bf[:, m, :], ps[:])
                    # gx^T = Wx^T @ Xa^T (+b) per direction
                    for di, d in enumerate("fb"):
                        for m in range(NM):
                            ps = pspool.tile([128, T], F32, tag="psA")
                            for k in range(KD):
                                nc.tensor.matmul(
                                    ps[:],
                                    wx_sb[d][:, k * G4 + m * 128:k * G4 + (m + 1) * 128],
                                    xat_bf[:, k, :], start=(k == 0), stop=(k == KD - 1))
                            nc.vector.tensor_scalar_add(
                                gates[d][:].rearrange(
                                    "p (t m s) -> p t m s",
                                    m=NM, s=bl)[:, :, m, ds(s, 1)],
                                ps[:].rearrange("p (t o) -> p t o", o=1),
                                bias_sb[:, di * NM + m:di * NM + m + 1])

            # ---------------- Phase R: BiLSTM recurrence (HW loop) ----------------
            with (
                tc.tile_pool(name="zps", bufs=4, space="PSUM") as zpool,
                tc.tile_pool(name="zsb", bufs=4) as zsbp,
                tc.tile_pool(name="sg", bufs=4) as sgp,
            ):
                with tc.For_i(0, T, 1) as i:
                    for d in "fb":
                        if d == "f":
                            roff = i * B8
                            woff = (i + 1) * B8
                            gcol = i * GB
                        else:
                            roff = (T - i) * B8
                            woff = (T - 1 - i) * B8
                            gcol = (T - 1 - i) * GB
                        hprev = sgp.tile([128, B8], BF16, tag=f"hprev_{d}")
                        nc.vector.tensor_copy(
                            hprev[:], hstore[d][:, ds(roff, B8)])
                        z_ps = zpool.tile([128, GB], F32, tag="z_ps")
                        for m in range(NM):
                            for j in range(KH):
                                nc.tensor.matmul(
                                    z_ps[:, m * bl:(m + 1) * bl],
                                    wh_sb[d][:, j * G4 + m * 128:j * G4 + (m + 1) * 128],
                                    hprev[:, j * bl:(j + 1) * bl],
                                    start=(j == 0), stop=(j == KH - 1))
                        z_sb = zsbp.tile([128, GB], F32, tag="z_sb")
                        nc.vector.scalar_tensor_tensor(
                            z_sb[:], z_ps[:], 1.0, gates[d][:, ds(gcol, GB)],
                            ALU.bypass, ALU.add)
                        sg = sgp.tile([128, GB], F32, tag="sg")
                        nc.scalar.activation(
                            sg[:, :6 * bl], z_sb[:, :6 * bl], AF.Sigmoid)
                        nc.scalar.activation(
                            sg[:, 6 * bl:], z_sb[:, 6 * bl:], AF.Tanh)
                        u = sgp.tile([128, B8], F32, tag="u")
                        nc.vector.scalar_tensor_tensor(
                            u[:], sg[:, :B8], 1.0, sg[:, 6 * bl:], ALU.bypass, ALU.mult)
                        q = sgp.tile([128, B8], F32, tag="q")
                        nc.vector.scalar_tensor_tensor(
                            q[:], sg[:, B8:2 * B8], 1.0, cstate[d][:],
                            ALU.bypass, ALU.mult)
                        nc.vector.scalar_tensor_tensor(
                            cstate[d][:], u[:], 1.0, q[:], ALU.bypass, ALU.add)
                        ct = sgp.tile([128, B8], F32, tag="ct")
                        nc.scalar.activation(ct[:], cstate[d][:], AF.Tanh)
                        nc.vector.scalar_tensor_tensor(
                            hstore[d][:, ds(woff, B8)],
                            sg[:, 2 * B8:3 * B8], 1.0, ct[:], ALU.bypass, ALU.mult)

            # ---------------- Phase C: classifier + softmax ----------------
            with (
                tc.tile_pool(name="cps", bufs=4, space="PSUM") as cpsp,
                tc.tile_pool(name="csb", bufs=4) as csbp,
            ):
                NTB = T // 128
                out_flat = out_d[:].rearrange("s t v -> (s t) v")
                # [p, c, t] views: c = within-slot column (j*bl + sample),
                # t = slot index (stride B8)
                vw = {d: hstore[d][:].rearrange("p (t c) -> p c t", c=B8)
                      for d in "fb"}
                with tc.For_i(0, bl, 1) as s:
                    for m in range(NTB):
                        hst = csbp.tile([128, 4, 128], BF16, tag="hst")
                        for k in range(4):
                            # fwd h(t) lives at slot t+1, bwd h(t) at slot t
                            d = "f" if k < 2 else "b"
                            t0 = m * 128 + (1 if k < 2 else 0)
                            nc.vector.tensor_copy(
                                hst[:, k:k + 1, :],
                                vw[d][:, ds((k % 2) * bl + s, 1), t0:t0 + 128])
                        ps = cpsp.tile([128, V + 1], F32, tag="psC")
                        for k in range(4):
                            nc.tensor.matmul(
                                ps[:], hst[:, k, :],
                                wc_sb[:, k * (V + 1):(k + 1) * (V + 1)],
                                start=(k == 0), stop=(k == 3))
                        lg = csbp.tile([128, V + 1], F32, tag="lg")
                        nc.vector.scalar_tensor_tensor(
                            lg[:], ps[:], 1.0, bc_sb[:], ALU.bypass, ALU.add)
                        e = csbp.tile([128, V + 1], F32, tag="e")
                        esum = csbp.tile([128, 1], F32, tag="esum")
                        nc.scalar.activation(e[:], lg[:], AF.Exp,
                                             accum_out=esum[:])
                        er = csbp.tile([128, 1], F32, tag="er")
                        nc.vector.reciprocal(er[:], esum[:])
                        o = csbp.tile([128, V + 1], BF16, tag="o")
                        nc.vector.tensor_scalar_mul(o[:], e[:], er[:])
                        nc.sync.dma_start(
                            out_flat[ds(s * T + m * 128, 128), :], o[:])

        _rep.close()

    nc.compile()
    return nc


_QBUF = {}


def _quant_x(X):
    """X -> (shipped array, Wx scale).  int8 mode: global symmetric quant."""
    X = np.asarray(X, np.float32)
    if not X_INT8:
        return X.astype(BF16NP), 1.0
    g = max(float(X.max()), -float(X.min()), 1e-30)
    if _QBUF.get("shape") != X.shape:
        _QBUF["shape"] = X.shape
        _QBUF["f"] = np.empty(X.shape, np.float32)
        _QBUF["i"] = np.empty(X.shape, np.int8)
    f, q = _QBUF["f"], _QBUF["i"]
    np.multiply(X, 127.0 / g, out=f)
    np.rint(f, out=f)
    np.copyto(q, f, casting="unsafe")
    return q, g / 127.0


def _host_inputs(Wx_f, Wh_f, b_f, Wx_b, Wh_b, b_b, Wc, bc,
                 use_allgather=USE_ALLGATHER, wx_scale=1.0):
    com = {}
    bcrep = np.broadcast_to(np.asarray(bc, np.float32), (128, V + 1))
    wc_pack = np.zeros((128, VP), BF16NP)
    wc_pack[:, :4 * (V + 1)] = _host_stationary(
        np.asarray(Wc, np.float32)).astype(BF16NP)

    bias_cols = np.zeros((128, 2 * NM), np.float32)
    wx_pack = {}
    wh_pack = {}
    for di, (wx, wh, b) in enumerate(
            [(Wx_f, Wh_f, b_f), (Wx_b, Wh_b, b_b)]):
        d = "fb"[di]
        wxp = _permute_gates(np.asarray(wx, np.float32) * wx_scale)
        whp = _permute_gates(np.asarray(wh, np.float32))
        bp = _permute_gates(np.asarray(b, np.float32))
        wx_pack[d] = _host_stationary(wxp).astype(BF16NP)
        wh_pack[d] = _host_stationary(whp).astype(BF16NP)
        bias_cols[:, di * NM:(di + 1) * NM] = bp.reshape(NM, 128).T

    if use_allgather:
        bias_bf = np.ascontiguousarray(bias_cols).view(BF16NP)
        bc_bf = np.ascontiguousarray(bcrep).view(BF16NP)
        shards = []
        for r in range(NCORES):
            shards.append(np.concatenate([
                wx_pack["f"][:, r * WXS:(r + 1) * WXS],
                wx_pack["b"][:, r * WXS:(r + 1) * WXS],
                wh_pack["f"][:, r * WHS:(r + 1) * WHS],
                wh_pack["b"][:, r * WHS:(r + 1) * WHS],
                wc_pack[:, r * WCS:(r + 1) * WCS],
                np.zeros((128, 1), BF16NP), bias_bf, bc_bf,
            ], axis=1))
        com["_WSH_SHARDS"] = shards
    else:
        com["BIAS"] = bias_cols
        com["BCREP"] = bcrep.copy()
        com["WXf"] = wx_pack["f"]
        com["WXb"] = wx_pack["b"]
        com["WHf"] = wh_pack["f"]
        com["WHb"] = wh_pack["b"]
        com["WC"] = wc_pack
    return com


def _make_in_maps(com, Xq):
    shards = com.pop("_WSH_SHARDS", None)
    in_maps = []
    for i in range(NCORES):
        m = dict(com)
        m["XBF"] = Xq[i * BL:(i + 1) * BL]
        if shards is not None:
            m["PK"] = shards[i]
        in_maps.append(m)
    if shards is not None:
        com["_WSH_SHARDS"] = shards
    return in_maps


_CACHE = {}


def kernel(X, Wx_f, Wh_f, b_f, Wx_b, Wh_b, b_b, Wc, bc,
           label=None, inputlength=None, labellength=None):
    key = "prog"
    if key not in _CACHE:
        _CACHE[key] = build_program()
    nc = _CACHE[key]
    Xq, wxs = _quant_x(X)
    com = _host_inputs(Wx_f, Wh_f, b_f, Wx_b, Wh_b, b_b, Wc, bc, wx_scale=wxs)
    in_maps = _make_in_maps(com, Xq)
    res = bass_utils.run_bass_kernel_spmd(nc, in_maps, core_ids=list(range(NCORES)))
    out = np.concatenate([r["OUT"] for r in res.results], axis=0)
    return np.ascontiguousarray(out.astype(np.float32))


if __name__ == "__main__":
    import reference
    ins = {k: np.asarray(v) for k, v in reference.setup_inputs().items()}
    got = kernel(**ins)
    want = np.asarray(reference.reference(**ins))
    err = np.abs(got - want).max() / np.abs(want).max()
    print("abs-rel err:", err)



# revision 39
# speedup vs baseline: 1.7380x; 1.7380x over previous
"""Trainium2 Bass kernel: cosine-attention + positional-adjacency mix + BiLSTM + softmax classifier.

Model (per sample, reference semantics):
    Xn   = X / ||X||_row
    Xa   = (Xn Xn^T) @ A_D @ X          (A_D = row-normalized exp(-|i-j|/8), constant)
    h    = BiLSTM(Xa)                    (fwd + bwd, H=256)
    out  = softmax(h @ Wc + bc)

Strategy: data-parallel over batch across 8 cores. All device matmuls in bf16
with fp32 PSUM accumulation; the feedforward runs in "transposed" layout so
the LSTM gate math operates on 128-partition tiles (see phase comments).

Wall-clock-per-call engineering (the metric is wall time of kernel(); the
axon tunnel to the devices has ~80 ms per-op latency and ~30-40 MB/s
aggregate bandwidth, so bytes moved on the wire dominate -- device compute
measures as negligible next to the transfers):
  - The shard_map executable is AOT-compiled ONCE (fast-dispatch, effects
    suppressed) and cached; repeat calls skip re-trace/re-lower/reload.
    This replaces per-call run_bass_kernel_spmd, which rebuilds the jit
    wrapper (and re-serializes the BIR) on every invocation.
  - Weights ship once (content-hashed): a single packed bf16 blob (PK),
    replicated to all 8 cores, stays device-resident across calls. The X
    int8 quantization scale is folded into Wx inside that blob, so X
    re-uploads never force weight re-uploads. No collectives in the
    program (a full weight copy sits on every core).
  - X ships as globally-scaled int8 (8.4 MB instead of 33.6 MB f32); the
    scale cancels in the cosine normalization and is folded into Wx for
    the value path. Sub-8-bit X was measured and rejected: int7/int6
    X-quant alone costs 1.5e-2/2.9e-2 rel err vs the 2e-2 gate.
  - Per-core upload pipeline: 8 worker threads each quantize their core's
    1 MB chunk then device_put it to that core alone (single-device puts
    from distinct threads pipeline at full link bandwidth; one big sharded
    put works too, but per-core puts overlap quantization with the wire).
    The shards are assembled zero-copy via
    make_array_from_single_device_arrays.
  - Output ships as uint8 (97 B/row instead of 97*2 B bf16): the device
    writes q = round(e * 254/max(e)); the host recovers the softmax as
    q/sum(q) -- rows sum to 1, so no scale bytes are needed. Fetch+decode
    run per-shard in threads.
  - The zero "donation" buffer for OUT is a device-resident dummy (the
    kernel writes every OUT element, so no per-call zeros upload and no
    donation -- the buffer is reused forever).
"""

import os
import threading
import zlib
from concurrent.futures import ThreadPoolExecutor

import numpy as np
import ml_dtypes

os.environ.setdefault("JAX_COMPILATION_CACHE_DIR", "/tmp/jaxcache")
import jax
try:
    jax.config.update("jax_compilation_cache_dir",
                      os.environ["JAX_COMPILATION_CACHE_DIR"])
    jax.config.update("jax_persistent_cache_min_entry_size_bytes", -1)
    jax.config.update("jax_persistent_cache_min_compile_time_secs", 0)
except Exception:
    pass

from jax.sharding import Mesh, PartitionSpec, NamedSharding
from jax.experimental.shard_map import shard_map

import concourse.bass as bass
from concourse.bass import ds
import concourse.mybir as mybir
import concourse.bacc as bacc
import concourse.tile as tile
from concourse import bass2jax

F32 = mybir.dt.float32
BF16 = mybir.dt.bfloat16
U8 = mybir.dt.uint8
I8 = mybir.dt.int8
AF = mybir.ActivationFunctionType
ALU = mybir.AluOpType
BF16NP = ml_dtypes.bfloat16

B_ALL, T_FULL, D, H, V = 32, 512, 512, 256, 96
SIGMA = 8.0
NCORES = 8
G4 = 4 * H                    # 1024 gate dims (permuted order i,f,o,g)
NM = G4 // 128                # 8 gate m-tiles
KD = D // 128                 # 4
KH = H // 128                 # 2
VP = 392                      # WC cols padded 4*(V+1)=388 -> 392
VOUT = V + 1                  # uint8 out row: 97 q values (host renormalizes)

# packed replicated weight blob (bf16 cols): WXf | WXb | WHf | WHb | WC | BIAS | BCREP
_WXC = KD * G4                # 4096
_WHC = KH * G4                # 2048
_C_WXF = 0
_C_WXB = _C_WXF + _WXC
_C_WHF = _C_WXB + _WXC
_C_WHB = _C_WHF + _WHC
_C_WC = _C_WHB + _WHC
_C_BIAS = _C_WC + VP          # f32 section (even offset)
_C_BC = _C_BIAS + 2 * (2 * NM)
PKC = _C_BC + 2 * (V + 1)     # 12906

BL_WAVE = 4                   # samples/core/execution (monolithic: one exec)
OUT_ROUND = float(os.environ.get("KROUND", "0.0"))  # f32->u8 rounding offset
N_QTHREADS = 4                # host-side quant/decode parallelism (= bl)


def _host_stationary(q):
    """[R, C] -> [128, (R//128)*C]; k-th col-block = rows k*128:(k+1)*128."""
    r, c = q.shape
    return np.ascontiguousarray(
        q.reshape(r // 128, 128, c).transpose(1, 0, 2).reshape(128, (r // 128) * c)
    )


def _permute_gates(w):
    """Reorder last-dim gate blocks [i,f,g,o] -> [i,f,o,g]."""
    i, f, g, o = np.split(w, 4, axis=-1)
    return np.concatenate([i, f, o, g], axis=-1)


def build_program(bl=BL_WAVE, n_devices=NCORES):
    T = T_FULL
    KT = T // 128
    nc = bacc.Bacc(
        "TRN2", target_bir_lowering=False, debug=False, enable_asserts=False,
        num_devices=n_devices,
    )

    x_in = nc.dram_tensor("XBF", [bl, T, D], I8, kind="ExternalInput")
    pk_in = nc.dram_tensor("PK", [128, PKC], BF16, kind="ExternalInput")
    out_d = nc.dram_tensor("OUT", [bl, T, VOUT], U8, kind="ExternalOutput")

    bias_in = pk_in[:, _C_BIAS:_C_BIAS + 4 * NM].bitcast(F32)
    bc_in = pk_in[:, _C_BC:PKC].bitcast(F32)

    B8 = 2 * bl    # h-cols per hstore slot
    GB = NM * bl   # z free cols

    with tile.TileContext(nc) as tc:
        with (
            tc.tile_pool(name="const", bufs=1) as cpool,
            tc.tile_pool(name="gates", bufs=1) as gpool,
            tc.tile_pool(name="state", bufs=1) as spool,
        ):
            wx_sb = {}
            wh_sb = {}
            for di, d in enumerate("fb"):
                wx_sb[d] = cpool.tile([128, _WXC], BF16, name=f"wx_{d}")
                nc.sync.dma_start(
                    wx_sb[d][:],
                    pk_in[:, (_C_WXF, _C_WXB)[di]:(_C_WXF, _C_WXB)[di] + _WXC])
                wh_sb[d] = cpool.tile([128, _WHC], BF16, name=f"wh_{d}")
                nc.sync.dma_start(
                    wh_sb[d][:],
                    pk_in[:, (_C_WHF, _C_WHB)[di]:(_C_WHF, _C_WHB)[di] + _WHC])
            wc_sb = cpool.tile([128, VP], BF16)
            nc.sync.dma_start(wc_sb[:], pk_in[:, _C_WC:_C_WC + VP])
            bias_sb = cpool.tile([128, 2 * NM], F32)
            nc.sync.dma_start(bias_sb[:], bias_in)
            bc_sb = cpool.tile([128, V + 1], F32)
            nc.sync.dma_start(bc_sb[:], bc_in)

            # A_D generated on device: ad_sb[p, k*T+c] = AD[k*128+p, c]
            ad_sb = cpool.tile([128, KT * T], BF16)
            with tc.tile_pool(name="adgen", bufs=2) as agp:
                for k in range(KT):
                    df = agp.tile([128, T], F32, tag="df")
                    nc.gpsimd.iota(df[:], pattern=[[1, T]], base=-(k * 128),
                                   channel_multiplier=-1,
                                   allow_small_or_imprecise_dtypes=True)
                    ab = agp.tile([128, T], F32, tag="ab")
                    nc.scalar.activation(ab[:], df[:], AF.Abs)
                    ex = agp.tile([128, T], F32, tag="ex")
                    ssum = agp.tile([128, 1], F32, tag="ssum")
                    nc.scalar.activation(ex[:], ab[:], AF.Exp,
                                         scale=-1.0 / SIGMA, accum_out=ssum[:])
                    rs = agp.tile([128, 1], F32, tag="rs")
                    nc.vector.reciprocal(rs[:], ssum[:])
                    nc.vector.tensor_scalar_mul(
                        ad_sb[:, k * T:(k + 1) * T], ex[:], rs[:])

            gates = {d: gpool.tile([128, NM * bl * T], BF16, name=f"gates_{d}")
                     for d in "fb"}
            hstore = {d: spool.tile([128, (T + 1) * B8], BF16, name=f"hstore_{d}")
                      for d in "fb"}
            cstate = {d: spool.tile([128, B8], F32, name=f"cstate_{d}") for d in "fb"}
            for d in "fb":
                z0 = 0 if d == "f" else T
                nc.vector.memset(hstore[d][:, z0 * B8:(z0 + 1) * B8], 0.0)
                nc.vector.memset(cstate[d][:], 0.0)

            # ---------------- Phase A: feedforward per sample ----------------
            with (
                tc.tile_pool(name="xb", bufs=2) as xbp,
                tc.tile_pool(name="mats", bufs=2) as mpool,
                tc.tile_pool(name="small", bufs=4) as smpool,
                tc.tile_pool(name="ps", bufs=4, space="PSUM") as pspool,
            ):
                with tc.For_i(0, bl, 1) as s:
                    x_bf = xbp.tile([128, KT, D], BF16, tag="x_bf")
                    x8 = xbp.tile([128, KT, D], I8, tag="x8")
                    nc.sync.dma_start(
                        x8[:].rearrange("p (o k) d -> p o k d", o=1),
                        x_in[ds(s, 1)].rearrange("o (k p) d -> p o k d", p=128))
                    nc.vector.tensor_copy(x_bf[:], x8[:])
                    xn_bf = xbp.tile([128, KT, D], BF16, tag="xn_bf")
                    xnt_bf = xbp.tile([128, KD, T], BF16, tag="xnt_bf")
                    dump = smpool.tile([128, D], BF16, tag="dump")
                    for k in range(KT):
                        ss = smpool.tile([128, 1], F32, tag="ss")
                        nc.scalar.activation(dump[:], x_bf[:, k, :], AF.Square,
                                             accum_out=ss[:])
                        sn = smpool.tile([128, 1], F32, tag="sn")
                        nc.scalar.activation(sn[:], ss[:], AF.Sqrt)
                        rn = smpool.tile([128, 1], F32, tag="rn")
                        nc.vector.reciprocal(rn[:], sn[:])
                        nc.vector.tensor_scalar_mul(xn_bf[:, k, :], x_bf[:, k, :], rn[:])
                    # Xn^T via DMA block transposes
                    for ti in range(KT):
                        for dj in range(KD):
                            nc.sync.dma_start_transpose(
                                xnt_bf[:, dj, ti * 128:(ti + 1) * 128],
                                xn_bf[:, ti, dj * 128:(dj + 1) * 128])
                    # A_S = Xn Xn^T   [T, T]
                    as_bf = mpool.tile([128, KT, T], BF16, tag="as_bf")
                    for m in range(KT):
                        ps = pspool.tile([128, T], F32, tag="psA")
                        for k in range(KD):
                            nc.tensor.matmul(
                                ps[:], xnt_bf[:, k, m * 128:(m + 1) * 128],
                                xnt_bf[:, k, :], start=(k == 0), stop=(k == KD - 1))
                        nc.vector.tensor_copy(as_bf[:, m, :], ps[:])
                    # P = A_D^T @ A_S
                    p_bf = mpool.tile([128, KT, T], BF16, tag="p_bf")
                    for m in range(KT):
                        ps = pspool.tile([128, T], F32, tag="psA")
                        for k in range(KT):
                            nc.tensor.matmul(
                                ps[:], ad_sb[:, k * T + m * 128:k * T + (m + 1) * 128],
                                as_bf[:, k, :], start=(k == 0), stop=(k == KT - 1))
                        nc.vector.tensor_copy(p_bf[:, m, :], ps[:])
                    # Xa^T = X^T @ P   [D, T]
                    xat_bf = mpool.tile([128, KD, T], BF16, tag="xat_bf")
                    for m in range(KD):
                        ps = pspool.tile([128, T], F32, tag="psA")
                        for k in range(KT):
                            nc.tensor.matmul(
                                ps[:], x_bf[:, k, m * 128:(m + 1) * 128],
                                p_bf[:, k, :], start=(k == 0), stop=(k == KT - 1))
                        nc.vector.tensor_copy(xat_bf[:, m, :], ps[:])
                    # gx^T = Wx^T @ Xa^T (+b) per direction
                    for di, d in enumerate("fb"):
                        for m in range(NM):
                            ps = pspool.tile([128, T], F32, tag="psA")
                            for k in range(KD):
                                nc.tensor.matmul(
                                    ps[:],
                                    wx_sb[d][:, k * G4 + m * 128:k * G4 + (m + 1) * 128],
                                    xat_bf[:, k, :], start=(k == 0), stop=(k == KD - 1))
                            nc.vector.tensor_scalar_add(
                                gates[d][:].rearrange(
                                    "p (t m s) -> p t m s",
                                    m=NM, s=bl)[:, :, m, ds(s, 1)],
                                ps[:].rearrange("p (t o) -> p t o", o=1),
                                bias_sb[:, di * NM + m:di * NM + m + 1])

            # ---------------- Phase R: BiLSTM recurrence (HW loop) ----------------
            with (
                tc.tile_pool(name="zps", bufs=4, space="PSUM") as zpool,
                tc.tile_pool(name="zsb", bufs=4) as zsbp,
                tc.tile_pool(name="sg", bufs=4) as sgp,
            ):
                with tc.For_i(0, T, 1) as i:
                    for d in "fb":
                        if d == "f":
                            roff = i * B8
                            woff = (i + 1) * B8
                            gcol = i * GB
                        else:
                            roff = (T - i) * B8
                            woff = (T - 1 - i) * B8
                            gcol = (T - 1 - i) * GB
                        hprev = sgp.tile([128, B8], BF16, tag=f"hprev_{d}")
                        nc.vector.tensor_copy(
                            hprev[:], hstore[d][:, ds(roff, B8)])
                        z_ps = zpool.tile([128, GB], F32, tag="z_ps")
                        for m in range(NM):
                            for j in range(KH):
                                nc.tensor.matmul(
                                    z_ps[:, m * bl:(m + 1) * bl],
                                    wh_sb[d][:, j * G4 + m * 128:j * G4 + (m + 1) * 128],
                                    hprev[:, j * bl:(j + 1) * bl],
                                    start=(j == 0), stop=(j == KH - 1))
                        z_sb = zsbp.tile([128, GB], F32, tag="z_sb")
                        nc.vector.scalar_tensor_tensor(
                            z_sb[:], z_ps[:], 1.0, gates[d][:, ds(gcol, GB)],
                            ALU.bypass, ALU.add)
                        sg = sgp.tile([128, GB], F32, tag="sg")
                        nc.scalar.activation(
                            sg[:, :6 * bl], z_sb[:, :6 * bl], AF.Sigmoid)
                        nc.scalar.activation(
                            sg[:, 6 * bl:], z_sb[:, 6 * bl:], AF.Tanh)
                        u = sgp.tile([128, B8], F32, tag="u")
                        nc.vector.scalar_tensor_tensor(
                            u[:], sg[:, :B8], 1.0, sg[:, 6 * bl:], ALU.bypass, ALU.mult)
                        q = sgp.tile([128, B8], F32, tag="q")
                        nc.vector.scalar_tensor_tensor(
                            q[:], sg[:, B8:2 * B8], 1.0, cstate[d][:],
                            ALU.bypass, ALU.mult)
                        nc.vector.scalar_tensor_tensor(
                            cstate[d][:], u[:], 1.0, q[:], ALU.bypass, ALU.add)
                        ct = sgp.tile([128, B8], F32, tag="ct")
                        nc.scalar.activation(ct[:], cstate[d][:], AF.Tanh)
                        nc.vector.scalar_tensor_tensor(
                            hstore[d][:, ds(woff, B8)],
                            sg[:, 2 * B8:3 * B8], 1.0, ct[:], ALU.bypass, ALU.mult)

            # ---------------- Phase C: classifier + softmax -> u8 ----------------
            with (
                tc.tile_pool(name="cps", bufs=4, space="PSUM") as cpsp,
                tc.tile_pool(name="csb", bufs=4) as csbp,
            ):
                NTB = T // 128
                out_flat = out_d[:].rearrange("s t v -> (s t) v")
                # [p, c, t] views: c = within-slot column (j*bl + sample),
                # t = slot index (stride B8)
                vw = {d: hstore[d][:].rearrange("p (t c) -> p c t", c=B8)
                      for d in "fb"}
                with tc.For_i(0, bl, 1) as s:
                    for m in range(NTB):
                        hst = csbp.tile([128, 4, 128], BF16, tag="hst")
                        for k in range(4):
                            # fwd h(t) lives at slot t+1, bwd h(t) at slot t
                            d = "f" if k < 2 else "b"
                            t0 = m * 128 + (1 if k < 2 else 0)
                            nc.vector.tensor_copy(
                                hst[:, k:k + 1, :],
                                vw[d][:, ds((k % 2) * bl + s, 1), t0:t0 + 128])
                        ps = cpsp.tile([128, V + 1], F32, tag="psC")
                        for k in range(4):
                            nc.tensor.matmul(
                                ps[:], hst[:, k, :],
                                wc_sb[:, k * (V + 1):(k + 1) * (V + 1)],
                                start=(k == 0), stop=(k == 3))
                        lg = csbp.tile([128, V + 1], F32, tag="lg")
                        nc.vector.scalar_tensor_tensor(
                            lg[:], ps[:], 1.0, bc_sb[:], ALU.bypass, ALU.add)
                        e = csbp.tile([128, V + 1], F32, tag="e")
                        esum = csbp.tile([128, 1], F32, tag="esum")
                        nc.scalar.activation(e[:], lg[:], AF.Exp,
                                             accum_out=esum[:])
                        # q = e*(254/max(e)) as u8; host renormalizes by sum(q)
                        # (softmax rows sum to 1, so no scale bytes needed)
                        rm = csbp.tile([128, 1], F32, tag="rm")
                        nc.vector.reduce_max(rm[:], e[:], axis=mybir.AxisListType.X)
                        rm254 = csbp.tile([128, 1], F32, tag="rm254")
                        nc.vector.tensor_scalar_mul(rm254[:], rm[:], 1.0 / 254.0)
                        rs254 = csbp.tile([128, 1], F32, tag="rs254")
                        nc.vector.reciprocal(rs254[:], rm254[:])
                        o = csbp.tile([128, VOUT], U8, tag="o")
                        nc.vector.tensor_scalar(
                            o[:, :V + 1], e[:], rs254[:], OUT_ROUND,
                            ALU.mult, ALU.add)
                        nc.sync.dma_start(
                            out_flat[ds(s * T + m * 128, 128), :], o[:])

    nc.compile()
    return nc


# ---------------------------------------------------------------------------
# Host-side runner: cached compiled executable + device-resident weights +
# threaded wave pipeline.
# ---------------------------------------------------------------------------

_CACHE = {}
_LOCK = threading.Lock()


def _hash_arrays(arrs):
    h = 0
    for a in arrs:
        a = np.ascontiguousarray(a)
        h = zlib.adler32(a.view(np.uint8).reshape(-1), h)
    return h


def _pack_pk(Wx_f, Wh_f, b_f, Wx_b, Wh_b, b_b, Wc, bc, wx_scale):
    """Full packed weight blob [128, PKC] bf16 (f32 sections bitcast)."""
    pk = np.empty((128, PKC), BF16NP)
    for di, (wx, wh) in enumerate([(Wx_f, Wh_f), (Wx_b, Wh_b)]):
        wxp = _permute_gates(np.asarray(wx, np.float32) * wx_scale)
        whp = _permute_gates(np.asarray(wh, np.float32))
        pk[:, (_C_WXF, _C_WXB)[di]:(_C_WXF, _C_WXB)[di] + _WXC] = \
            _host_stationary(wxp).astype(BF16NP)
        pk[:, (_C_WHF, _C_WHB)[di]:(_C_WHF, _C_WHB)[di] + _WHC] = \
            _host_stationary(whp).astype(BF16NP)
    wc_pack = np.zeros((128, VP), BF16NP)
    wc_pack[:, :4 * (V + 1)] = _host_stationary(
        np.asarray(Wc, np.float32)).astype(BF16NP)
    pk[:, _C_WC:_C_WC + VP] = wc_pack
    bias_cols = np.zeros((128, 2 * NM), np.float32)
    for di, b in enumerate([b_f, b_b]):
        bp = _permute_gates(np.asarray(b, np.float32))
        bias_cols[:, di * NM:(di + 1) * NM] = bp.reshape(NM, 128).T
    pk[:, _C_BIAS:_C_BIAS + 4 * NM] = bias_cols.view(BF16NP)
    bcrep = np.ascontiguousarray(
        np.broadcast_to(np.asarray(bc, np.float32), (128, V + 1)))
    pk[:, _C_BC:PKC] = bcrep.view(BF16NP)
    return pk


def _setup_runner():
    """Build program, jit it, warm the compile. Cached."""
    if "runner" in _CACHE:
        return _CACHE["runner"]
    nc = build_program(BL_WAVE)
    bass2jax.install_neuronx_cc_hook()

    partition_name = nc.partition_id_tensor.name if nc.partition_id_tensor else None
    in_names, out_names, out_avals = [], [], []
    for alloc in nc.m.functions[0].allocations:
        if not isinstance(alloc, mybir.MemoryLocationSet):
            continue
        name = alloc.memorylocations[0].name
        if alloc.kind == "ExternalInput":
            if name != partition_name:
                in_names.append(name)
        elif alloc.kind == "ExternalOutput":
            out_names.append(name)
            out_avals.append(jax.core.ShapedArray(
                tuple(alloc.tensor_shape), mybir.dt.np(alloc.dtype)))
    n_params = len(in_names)
    bind_names = in_names + out_names + ([partition_name] if partition_name else [])

    def _body(*args_):
        operands = list(args_)
        if partition_name is not None:
            operands.append(bass2jax.partition_id_tensor())
        outs = bass2jax._bass_exec_p.bind(
            *operands, out_avals=tuple(out_avals), in_names=tuple(bind_names),
            out_names=tuple(out_names), lowering_input_output_aliases=(),
            sim_require_finite=True, sim_require_nnan=True, nc=nc)
        return tuple(outs)

    mesh = Mesh(np.asarray(jax.devices()[:NCORES]), ("core",))
    n_outs = len(out_names)
    sm = shard_map(_body, mesh=mesh,
                   in_specs=(PartitionSpec("core"),) * (n_params + n_outs),
                   out_specs=(PartitionSpec("core"),) * n_outs,
                   check_rep=False)
    structs = (
        jax.ShapeDtypeStruct((B_ALL, T_FULL, D), np.int8),
        jax.ShapeDtypeStruct((NCORES * 128, PKC), ml_dtypes.bfloat16),
        jax.ShapeDtypeStruct((B_ALL, T_FULL, VOUT), np.uint8))
    # AOT compile with BassEffect suppressed -> C++ fast-path dispatch
    fn = bass2jax.fast_dispatch_compile(
        lambda: jax.jit(sm, keep_unused=True).lower(*structs).compile())
    sh = NamedSharding(mesh, PartitionSpec("core"))
    Z_dev = jax.device_put(
        np.zeros((NCORES * BL_WAVE, T_FULL, VOUT), np.uint8), sh)
    runner = {"fn": fn, "sh": sh, "Z": Z_dev, "in_names": in_names}
    _CACHE["runner"] = runner
    return runner


def _ensure_weights(runner, X, Wx_f, Wh_f, b_f, Wx_b, Wh_b, b_b, Wc, bc):
    """Device-resident packed weights keyed by content hash; returns (PK_dev, g0)."""
    wh = _hash_arrays([Wx_f, Wh_f, b_f, Wx_b, Wh_b, b_b, Wc, bc])
    st = _CACHE.get("weights")
    if st is not None and st["hash"] == wh:
        # g0 stays; quant-time extrema check handles X outgrowing it
        return st["PK_dev"], st["g0"]
    g0 = max(float(np.max(X)), -float(np.min(X)), 1e-30)
    pk = _pack_pk(Wx_f, Wh_f, b_f, Wx_b, Wh_b, b_b, Wc, bc, g0 / 127.0)
    pk_rep = np.ascontiguousarray(
        np.broadcast_to(pk[None], (NCORES, 128, PKC))).reshape(NCORES * 128, PKC)
    PK_dev = jax.device_put(pk_rep, runner["sh"])
    PK_dev.block_until_ready()
    _CACHE["weights"] = {"hash": wh, "g0": g0, "PK_dev": PK_dev}
    return PK_dev, g0


_QSTATE = {}


def _quant_put(X, scale, sh, pool):
    """Quantize per-core chunks and upload each to its device as it's ready.

    Thread c quantizes X[4c:4c+4] (1 MB int8) then device_puts it to core
    c's device alone, so later chunks' quantization overlaps earlier
    chunks' wire transfer; single-device puts issued from distinct threads
    pipeline at full link bandwidth. The 8 shards are assembled zero-copy
    into one global array for the compiled call.

    Returns (X_dev, fmax, fmin) with fmax/fmin extrema of X*scale.
    """
    if "q" not in _QSTATE:
        _QSTATE["q"] = [np.empty((BL_WAVE, T_FULL, D), np.int8)
                        for _ in range(NCORES)]
        _QSTATE["f"] = [np.empty((BL_WAVE, T_FULL, D), np.float32)
                        for _ in range(NCORES)]
        _QSTATE["devs"] = jax.devices()[:NCORES]

    def work(c):
        f = _QSTATE["f"][c]
        q = _QSTATE["q"][c]
        np.multiply(X[c * BL_WAVE:(c + 1) * BL_WAVE], scale, out=f)
        mx, mn = float(np.max(f)), float(np.min(f))
        np.rint(f, out=f)
        np.copyto(q, f, casting="unsafe")
        d = jax.device_put(q, _QSTATE["devs"][c])
        return d, mx, mn

    res = list(pool.map(work, range(NCORES)))
    X_dev = jax.make_array_from_single_device_arrays(
        (B_ALL, T_FULL, D), sh, [r[0] for r in res])
    fmax = max(r[1] for r in res)
    fmin = min(r[2] for r in res)
    return X_dev, fmax, fmin


def _decode_all(buf, out, pool):
    """buf [B,512,97] u8 -> out f32 [B,512,97]: p = q / sum(q) per row."""
    nb = buf.shape[0]
    nchunk = max(1, nb // N_QTHREADS)

    def work(lo):
        hi = min(lo + nchunk, nb)
        qv = buf[lo:hi].astype(np.float32)
        s = qv.sum(-1, keepdims=True)
        np.reciprocal(s, out=s)
        np.multiply(qv, s, out=out[lo:hi])

    if pool is None:
        for lo in range(0, nb, nchunk):
            work(lo)
    else:
        list(pool.map(work, range(0, nb, nchunk)))


def kernel(X, Wx_f, Wh_f, b_f, Wx_b, Wh_b, b_b, Wc, bc,
           label=None, inputlength=None, labellength=None):
    X = np.asarray(X, np.float32)
    with _LOCK:
        runner = _setup_runner()
        PK_dev, g0 = _ensure_weights(
            runner, X, Wx_f, Wh_f, b_f, Wx_b, Wh_b, b_b, Wc, bc)
        if "pool" not in _CACHE:
            _CACHE["pool"] = ThreadPoolExecutor(max_workers=NCORES)
        pool = _CACHE["pool"]
        sh = runner["sh"]
        X_dev, fmax, fmin = _quant_put(X, 127.0 / g0, sh, pool)
        if fmax > 127.49 or fmin < -127.49:
            # X exceeds the cached calibration: re-fold weights with new g
            g0 = max(fmax, -fmin) * g0 / 127.0
            st = _CACHE["weights"]
            pk = _pack_pk(Wx_f, Wh_f, b_f, Wx_b, Wh_b, b_b, Wc, bc, g0 / 127.0)
            pk_rep = np.ascontiguousarray(
                np.broadcast_to(pk[None], (NCORES, 128, PKC))
            ).reshape(NCORES * 128, PKC)
            PK_dev = jax.device_put(pk_rep, sh)
            PK_dev.block_until_ready()
            st.update(g0=g0, PK_dev=PK_dev)
            X_dev, _, _ = _quant_put(X, 127.0 / g0, sh, pool)
        fn, Z = runner["fn"], runner["Z"]
        out = np.empty((B_ALL, T_FULL, V + 1), np.float32)
        outs = fn(X_dev, PK_dev, Z)
        # fetch per-core shards from threads (overlaps wire + decode)
        shards = outs[0].addressable_shards

        def fetch_dec(i):
            shd = shards[i]
            lo = shd.index[0].start or 0
            buf = np.asarray(shd.data)
            _decode_all(buf, out[lo:lo + buf.shape[0]], None)

        list(pool.map(fetch_dec, range(NCORES)))
    return out


if __name__ == "__main__":
    import reference
    ins = {k: np.asarray(v) for k, v in reference.setup_inputs().items()}
    got = kernel(**ins)
    want = np.asarray(reference.reference(**ins))
    err = np.abs(got - want).max() / np.abs(want).max()
    print("abs-rel err:", err)


# revision 41
# speedup vs baseline: 1.7775x; 1.0228x over previous
"""Trainium2 Bass kernel: cosine-attention + positional-adjacency mix + BiLSTM + softmax classifier.

Model (per sample, reference semantics):
    Xn   = X / ||X||_row
    Xa   = (Xn Xn^T) @ A_D @ X          (A_D = row-normalized exp(-|i-j|/8), constant)
    h    = BiLSTM(Xa)                    (fwd + bwd, H=256)
    out  = softmax(h @ Wc + bc)

Strategy: data-parallel over batch across 8 cores. All device matmuls in bf16
with fp32 PSUM accumulation; the feedforward runs in "transposed" layout so
the LSTM gate math operates on 128-partition tiles (see phase comments).

Wall-clock-per-call engineering (the metric is wall time of kernel(); the
axon tunnel to the devices has ~80 ms per-op latency and ~30-40 MB/s
aggregate bandwidth, so bytes moved on the wire dominate -- device compute
measures as negligible next to the transfers):
  - The shard_map executable is AOT-compiled ONCE (fast-dispatch, effects
    suppressed) and cached; repeat calls skip re-trace/re-lower/reload.
    This replaces per-call run_bass_kernel_spmd, which rebuilds the jit
    wrapper (and re-serializes the BIR) on every invocation.
  - Weights ship once (content-hashed): a single packed bf16 blob (PK),
    replicated to all 8 cores, stays device-resident across calls. The X
    int8 quantization scale is folded into Wx inside that blob, so X
    re-uploads never force weight re-uploads. No collectives in the
    program (a full weight copy sits on every core).
  - X ships as globally-scaled int8 (8.4 MB instead of 33.6 MB f32); the
    scale cancels in the cosine normalization and is folded into Wx for
    the value path. Sub-8-bit X was measured and rejected: int7/int6
    X-quant alone costs 1.5e-2/2.9e-2 rel err vs the 2e-2 gate.
  - Per-core upload pipeline: 8 worker threads each quantize their core's
    1 MB chunk then device_put it to that core alone (single-device puts
    from distinct threads pipeline at full link bandwidth; one big sharded
    put works too, but per-core puts overlap quantization with the wire).
    The shards are assembled zero-copy via
    make_array_from_single_device_arrays.
  - Output ships as uint8 (97 B/row instead of 97*2 B bf16): the device
    writes q = round(e * 254/max(e)); the host recovers the softmax as
    q/sum(q) -- rows sum to 1, so no scale bytes are needed. Fetch+decode
    run per-shard in threads.
  - The zero "donation" buffer for OUT is a device-resident dummy (the
    kernel writes every OUT element, so no per-call zeros upload and no
    donation -- the buffer is reused forever).
"""

import os
import threading
import zlib
from concurrent.futures import ThreadPoolExecutor

import numpy as np
import ml_dtypes

os.environ.setdefault("JAX_COMPILATION_CACHE_DIR", "/tmp/jaxcache")
import jax
try:
    jax.config.update("jax_compilation_cache_dir",
                      os.environ["JAX_COMPILATION_CACHE_DIR"])
    jax.config.update("jax_persistent_cache_min_entry_size_bytes", -1)
    jax.config.update("jax_persistent_cache_min_compile_time_secs", 0)
except Exception:
    pass

from jax.sharding import Mesh, PartitionSpec, NamedSharding
from jax.experimental.shard_map import shard_map

import concourse.bass as bass
from concourse.bass import ds
import concourse.mybir as mybir
import concourse.bacc as bacc
import concourse.tile as tile
from concourse import bass2jax

F32 = mybir.dt.float32
BF16 = mybir.dt.bfloat16
U8 = mybir.dt.uint8
I8 = mybir.dt.int8
AF = mybir.ActivationFunctionType
ALU = mybir.AluOpType
BF16NP = ml_dtypes.bfloat16

B_ALL, T_FULL, D, H, V = 32, 512, 512, 256, 96
SIGMA = 8.0
NCORES = 8
G4 = 4 * H                    # 1024 gate dims (permuted order i,f,o,g)
NM = G4 // 128                # 8 gate m-tiles
KD = D // 128                 # 4
KH = H // 128                 # 2
VP = 392                      # WC cols padded 4*(V+1)=388 -> 392
VOUT = V + 1                  # uint8 out row: 97 q values (host renormalizes)

# packed replicated weight blob (bf16 cols): WXf | WXb | WHf | WHb | WC | BIAS | BCREP
_WXC = KD * G4                # 4096
_WHC = KH * G4                # 2048
_C_WXF = 0
_C_WXB = _C_WXF + _WXC
_C_WHF = _C_WXB + _WXC
_C_WHB = _C_WHF + _WHC
_C_WC = _C_WHB + _WHC
_C_BIAS = _C_WC + VP          # f32 section (even offset)
_C_BC = _C_BIAS + 2 * (2 * NM)
PKC = _C_BC + 2 * (V + 1)     # 12906

BL_WAVE = 4                   # samples/core/execution (monolithic: one exec)
OUT_ROUND = float(os.environ.get("KROUND", "0.0"))  # f32->u8 rounding offset
N_QTHREADS = 4                # host-side quant/decode parallelism (= bl)


def _host_stationary(q):
    """[R, C] -> [128, (R//128)*C]; k-th col-block = rows k*128:(k+1)*128."""
    r, c = q.shape
    return np.ascontiguousarray(
        q.reshape(r // 128, 128, c).transpose(1, 0, 2).reshape(128, (r // 128) * c)
    )


def _permute_gates(w):
    """Reorder last-dim gate blocks [i,f,g,o] -> [i,f,o,g]."""
    i, f, g, o = np.split(w, 4, axis=-1)
    return np.concatenate([i, f, o, g], axis=-1)


def build_program(bl=BL_WAVE, n_devices=NCORES):
    T = T_FULL
    KT = T // 128
    nc = bacc.Bacc(
        "TRN2", target_bir_lowering=False, debug=False, enable_asserts=False,
        num_devices=n_devices,
    )

    x_in = nc.dram_tensor("XBF", [bl, T, D], I8, kind="ExternalInput")
    pk_in = nc.dram_tensor("PK", [128, PKC], BF16, kind="ExternalInput")
    out_d = nc.dram_tensor("OUT", [bl, T, VOUT], U8, kind="ExternalOutput")

    bias_in = pk_in[:, _C_BIAS:_C_BIAS + 4 * NM].bitcast(F32)
    bc_in = pk_in[:, _C_BC:PKC].bitcast(F32)

    B8 = 2 * bl    # h-cols per hstore slot
    GB = NM * bl   # z free cols

    with tile.TileContext(nc) as tc:
        with (
            tc.tile_pool(name="const", bufs=1) as cpool,
            tc.tile_pool(name="gates", bufs=1) as gpool,
            tc.tile_pool(name="state", bufs=1) as spool,
        ):
            wx_sb = {}
            wh_sb = {}
            for di, d in enumerate("fb"):
                wx_sb[d] = cpool.tile([128, _WXC], BF16, name=f"wx_{d}")
                nc.sync.dma_start(
                    wx_sb[d][:],
                    pk_in[:, (_C_WXF, _C_WXB)[di]:(_C_WXF, _C_WXB)[di] + _WXC])
                wh_sb[d] = cpool.tile([128, _WHC], BF16, name=f"wh_{d}")
                nc.sync.dma_start(
                    wh_sb[d][:],
                    pk_in[:, (_C_WHF, _C_WHB)[di]:(_C_WHF, _C_WHB)[di] + _WHC])
            wc_sb = cpool.tile([128, VP], BF16)
            nc.sync.dma_start(wc_sb[:], pk_in[:, _C_WC:_C_WC + VP])
            bias_sb = cpool.tile([128, 2 * NM], F32)
            nc.sync.dma_start(bias_sb[:], bias_in)
            bc_sb = cpool.tile([128, V + 1], F32)
            nc.sync.dma_start(bc_sb[:], bc_in)

            # A_D generated on device: ad_sb[p, k*T+c] = AD[k*128+p, c]
            ad_sb = cpool.tile([128, KT * T], BF16)
            with tc.tile_pool(name="adgen", bufs=2) as agp:
                for k in range(KT):
                    df = agp.tile([128, T], F32, tag="df")
                    nc.gpsimd.iota(df[:], pattern=[[1, T]], base=-(k * 128),
                                   channel_multiplier=-1,
                                   allow_small_or_imprecise_dtypes=True)
                    ab = agp.tile([128, T], F32, tag="ab")
                    nc.scalar.activation(ab[:], df[:], AF.Abs)
                    ex = agp.tile([128, T], F32, tag="ex")
                    ssum = agp.tile([128, 1], F32, tag="ssum")
                    nc.scalar.activation(ex[:], ab[:], AF.Exp,
                                         scale=-1.0 / SIGMA, accum_out=ssum[:])
                    rs = agp.tile([128, 1], F32, tag="rs")
                    nc.vector.reciprocal(rs[:], ssum[:])
                    nc.vector.tensor_scalar_mul(
                        ad_sb[:, k * T:(k + 1) * T], ex[:], rs[:])

            gates = {d: gpool.tile([128, NM * bl * T], BF16, name=f"gates_{d}")
                     for d in "fb"}
            hstore = {d: spool.tile([128, (T + 1) * B8], BF16, name=f"hstore_{d}")
                      for d in "fb"}
            cstate = {d: spool.tile([128, B8], F32, name=f"cstate_{d}") for d in "fb"}
            for d in "fb":
                z0 = 0 if d == "f" else T
                nc.vector.memset(hstore[d][:, z0 * B8:(z0 + 1) * B8], 0.0)
                nc.vector.memset(cstate[d][:], 0.0)

            # ---------------- Phase A: feedforward per sample ----------------
            with (
                tc.tile_pool(name="xb", bufs=2) as xbp,
                tc.tile_pool(name="mats", bufs=2) as mpool,
                tc.tile_pool(name="small", bufs=4) as smpool,
                tc.tile_pool(name="ps", bufs=4, space="PSUM") as pspool,
            ):
                with tc.For_i(0, bl, 1) as s:
                    x_bf = xbp.tile([128, KT, D], BF16, tag="x_bf")
                    x8 = xbp.tile([128, KT, D], I8, tag="x8")
                    nc.sync.dma_start(
                        x8[:].rearrange("p (o k) d -> p o k d", o=1),
                        x_in[ds(s, 1)].rearrange("o (k p) d -> p o k d", p=128))
                    nc.vector.tensor_copy(x_bf[:], x8[:])
                    xn_bf = xbp.tile([128, KT, D], BF16, tag="xn_bf")
                    xnt_bf = xbp.tile([128, KD, T], BF16, tag="xnt_bf")
                    dump = smpool.tile([128, D], BF16, tag="dump")
                    for k in range(KT):
                        ss = smpool.tile([128, 1], F32, tag="ss")
                        nc.scalar.activation(dump[:], x_bf[:, k, :], AF.Square,
                                             accum_out=ss[:])
                        sn = smpool.tile([128, 1], F32, tag="sn")
                        nc.scalar.activation(sn[:], ss[:], AF.Sqrt)
                        rn = smpool.tile([128, 1], F32, tag="rn")
                        nc.vector.reciprocal(rn[:], sn[:])
                        nc.vector.tensor_scalar_mul(xn_bf[:, k, :], x_bf[:, k, :], rn[:])
                    # Xn^T via DMA block transposes
                    for ti in range(KT):
                        for dj in range(KD):
                            nc.sync.dma_start_transpose(
                                xnt_bf[:, dj, ti * 128:(ti + 1) * 128],
                                xn_bf[:, ti, dj * 128:(dj + 1) * 128])
                    # A_S = Xn Xn^T   [T, T]
                    as_bf = mpool.tile([128, KT, T], BF16, tag="as_bf")
                    for m in range(KT):
                        ps = pspool.tile([128, T], F32, tag="psA")
                        for k in range(KD):
                            nc.tensor.matmul(
                                ps[:], xnt_bf[:, k, m * 128:(m + 1) * 128],
                                xnt_bf[:, k, :], start=(k == 0), stop=(k == KD - 1))
                        nc.vector.tensor_copy(as_bf[:, m, :], ps[:])
                    # P = A_D^T @ A_S
                    p_bf = mpool.tile([128, KT, T], BF16, tag="p_bf")
                    for m in range(KT):
                        ps = pspool.tile([128, T], F32, tag="psA")
                        for k in range(KT):
                            nc.tensor.matmul(
                                ps[:], ad_sb[:, k * T + m * 128:k * T + (m + 1) * 128],
                                as_bf[:, k, :], start=(k == 0), stop=(k == KT - 1))
                        nc.vector.tensor_copy(p_bf[:, m, :], ps[:])
                    # Xa^T = X^T @ P   [D, T]
                    xat_bf = mpool.tile([128, KD, T], BF16, tag="xat_bf")
                    for m in range(KD):
                        ps = pspool.tile([128, T], F32, tag="psA")
                        for k in range(KT):
                            nc.tensor.matmul(
                                ps[:], x_bf[:, k, m * 128:(m + 1) * 128],
                                p_bf[:, k, :], start=(k == 0), stop=(k == KT - 1))
                        nc.vector.tensor_copy(xat_bf[:, m, :], ps[:])
                    # gx^T = Wx^T @ Xa^T (+b) per direction
                    for di, d in enumerate("fb"):
                        for m in range(NM):
                            ps = pspool.tile([128, T], F32, tag="psA")
                            for k in range(KD):
                                nc.tensor.matmul(
                                    ps[:],
                                    wx_sb[d][:, k * G4 + m * 128:k * G4 + (m + 1) * 128],
                                    xat_bf[:, k, :], start=(k == 0), stop=(k == KD - 1))
                            nc.vector.tensor_scalar_add(
                                gates[d][:].rearrange(
                                    "p (t m s) -> p t m s",
                                    m=NM, s=bl)[:, :, m, ds(s, 1)],
                                ps[:].rearrange("p (t o) -> p t o", o=1),
                                bias_sb[:, di * NM + m:di * NM + m + 1])

            # ---------------- Phase R: BiLSTM recurrence (HW loop) ----------------
            with (
                tc.tile_pool(name="zps", bufs=4, space="PSUM") as zpool,
                tc.tile_pool(name="zsb", bufs=4) as zsbp,
                tc.tile_pool(name="sg", bufs=4) as sgp,
            ):
                with tc.For_i(0, T, 1) as i:
                    for d in "fb":
                        if d == "f":
                            roff = i * B8
                            woff = (i + 1) * B8
                            gcol = i * GB
                        else:
                            roff = (T - i) * B8
                            woff = (T - 1 - i) * B8
                            gcol = (T - 1 - i) * GB
                        hprev = sgp.tile([128, B8], BF16, tag=f"hprev_{d}")
                        nc.vector.tensor_copy(
                            hprev[:], hstore[d][:, ds(roff, B8)])
                        z_ps = zpool.tile([128, GB], F32, tag="z_ps")
                        for m in range(NM):
                            for j in range(KH):
                                nc.tensor.matmul(
                                    z_ps[:, m * bl:(m + 1) * bl],
                                    wh_sb[d][:, j * G4 + m * 128:j * G4 + (m + 1) * 128],
                                    hprev[:, j * bl:(j + 1) * bl],
                                    start=(j == 0), stop=(j == KH - 1))
                        z_sb = zsbp.tile([128, GB], F32, tag="z_sb")
                        nc.vector.scalar_tensor_tensor(
                            z_sb[:], z_ps[:], 1.0, gates[d][:, ds(gcol, GB)],
                            ALU.bypass, ALU.add)
                        sg = sgp.tile([128, GB], F32, tag="sg")
                        nc.scalar.activation(
                            sg[:, :6 * bl], z_sb[:, :6 * bl], AF.Sigmoid)
                        nc.scalar.activation(
                            sg[:, 6 * bl:], z_sb[:, 6 * bl:], AF.Tanh)
                        u = sgp.tile([128, B8], F32, tag="u")
                        nc.vector.scalar_tensor_tensor(
                            u[:], sg[:, :B8], 1.0, sg[:, 6 * bl:], ALU.bypass, ALU.mult)
                        q = sgp.tile([128, B8], F32, tag="q")
                        nc.vector.scalar_tensor_tensor(
                            q[:], sg[:, B8:2 * B8], 1.0, cstate[d][:],
                            ALU.bypass, ALU.mult)
                        nc.vector.scalar_tensor_tensor(
                            cstate[d][:], u[:], 1.0, q[:], ALU.bypass, ALU.add)
                        ct = sgp.tile([128, B8], F32, tag="ct")
                        nc.scalar.activation(ct[:], cstate[d][:], AF.Tanh)
                        nc.vector.scalar_tensor_tensor(
                            hstore[d][:, ds(woff, B8)],
                            sg[:, 2 * B8:3 * B8], 1.0, ct[:], ALU.bypass, ALU.mult)

            # ---------------- Phase C: classifier + softmax -> u8 ----------------
            with (
                tc.tile_pool(name="cps", bufs=4, space="PSUM") as cpsp,
                tc.tile_pool(name="csb", bufs=4) as csbp,
            ):
                NTB = T // 128
                out_flat = out_d[:].rearrange("s t v -> (s t) v")
                # [p, c, t] views: c = within-slot column (j*bl + sample),
                # t = slot index (stride B8)
                vw = {d: hstore[d][:].rearrange("p (t c) -> p c t", c=B8)
                      for d in "fb"}
                with tc.For_i(0, bl, 1) as s:
                    for m in range(NTB):
                        hst = csbp.tile([128, 4, 128], BF16, tag="hst")
                        for k in range(4):
                            # fwd h(t) lives at slot t+1, bwd h(t) at slot t
                            d = "f" if k < 2 else "b"
                            t0 = m * 128 + (1 if k < 2 else 0)
                            nc.vector.tensor_copy(
                                hst[:, k:k + 1, :],
                                vw[d][:, ds((k % 2) * bl + s, 1), t0:t0 + 128])
                        ps = cpsp.tile([128, V + 1], F32, tag="psC")
                        for k in range(4):
                            nc.tensor.matmul(
                                ps[:], hst[:, k, :],
                                wc_sb[:, k * (V + 1):(k + 1) * (V + 1)],
                                start=(k == 0), stop=(k == 3))
                        lg = csbp.tile([128, V + 1], F32, tag="lg")
                        nc.vector.scalar_tensor_tensor(
                            lg[:], ps[:], 1.0, bc_sb[:], ALU.bypass, ALU.add)
                        e = csbp.tile([128, V + 1], F32, tag="e")
                        esum = csbp.tile([128, 1], F32, tag="esum")
                        nc.scalar.activation(e[:], lg[:], AF.Exp,
                                             accum_out=esum[:])
                        # q = e*(254/max(e)) as u8; host renormalizes by sum(q)
                        # (softmax rows sum to 1, so no scale bytes needed)
                        rm = csbp.tile([128, 1], F32, tag="rm")
                        nc.vector.reduce_max(rm[:], e[:], axis=mybir.AxisListType.X)
                        rm254 = csbp.tile([128, 1], F32, tag="rm254")
                        nc.vector.tensor_scalar_mul(rm254[:], rm[:], 1.0 / 254.0)
                        rs254 = csbp.tile([128, 1], F32, tag="rs254")
                        nc.vector.reciprocal(rs254[:], rm254[:])
                        o = csbp.tile([128, VOUT], U8, tag="o")
                        nc.vector.tensor_scalar(
                            o[:, :V + 1], e[:], rs254[:], OUT_ROUND,
                            ALU.mult, ALU.add)
                        nc.sync.dma_start(
                            out_flat[ds(s * T + m * 128, 128), :], o[:])

    nc.compile()
    return nc


# ---------------------------------------------------------------------------
# Host-side runner: cached compiled executable + device-resident weights +
# threaded wave pipeline.
# ---------------------------------------------------------------------------

_CACHE = {}
_LOCK = threading.Lock()


def _hash_arrays(arrs):
    h = 0
    for a in arrs:
        a = np.ascontiguousarray(a)
        h = zlib.adler32(a.view(np.uint8).reshape(-1), h)
    return h


def _pack_pk(Wx_f, Wh_f, b_f, Wx_b, Wh_b, b_b, Wc, bc, wx_scale):
    """Full packed weight blob [128, PKC] bf16 (f32 sections bitcast)."""
    pk = np.empty((128, PKC), BF16NP)
    for di, (wx, wh) in enumerate([(Wx_f, Wh_f), (Wx_b, Wh_b)]):
        wxp = _permute_gates(np.asarray(wx, np.float32) * wx_scale)
        whp = _permute_gates(np.asarray(wh, np.float32))
        pk[:, (_C_WXF, _C_WXB)[di]:(_C_WXF, _C_WXB)[di] + _WXC] = \
            _host_stationary(wxp).astype(BF16NP)
        pk[:, (_C_WHF, _C_WHB)[di]:(_C_WHF, _C_WHB)[di] + _WHC] = \
            _host_stationary(whp).astype(BF16NP)
    wc_pack = np.zeros((128, VP), BF16NP)
    wc_pack[:, :4 * (V + 1)] = _host_stationary(
        np.asarray(Wc, np.float32)).astype(BF16NP)
    pk[:, _C_WC:_C_WC + VP] = wc_pack
    bias_cols = np.zeros((128, 2 * NM), np.float32)
    for di, b in enumerate([b_f, b_b]):
        bp = _permute_gates(np.asarray(b, np.float32))
        bias_cols[:, di * NM:(di + 1) * NM] = bp.reshape(NM, 128).T
    pk[:, _C_BIAS:_C_BIAS + 4 * NM] = bias_cols.view(BF16NP)
    bcrep = np.ascontiguousarray(
        np.broadcast_to(np.asarray(bc, np.float32), (128, V + 1)))
    pk[:, _C_BC:PKC] = bcrep.view(BF16NP)
    return pk


def _setup_runner():
    """Build program, jit it, warm the compile. Cached."""
    if "runner" in _CACHE:
        return _CACHE["runner"]
    nc = build_program(BL_WAVE)
    bass2jax.install_neuronx_cc_hook()

    partition_name = nc.partition_id_tensor.name if nc.partition_id_tensor else None
    in_names, out_names, out_avals = [], [], []
    for alloc in nc.m.functions[0].allocations:
        if not isinstance(alloc, mybir.MemoryLocationSet):
            continue
        name = alloc.memorylocations[0].name
        if alloc.kind == "ExternalInput":
            if name != partition_name:
                in_names.append(name)
        elif alloc.kind == "ExternalOutput":
            out_names.append(name)
            out_avals.append(jax.core.ShapedArray(
                tuple(alloc.tensor_shape), mybir.dt.np(alloc.dtype)))
    n_params = len(in_names)
    bind_names = in_names + out_names + ([partition_name] if partition_name else [])

    def _body(*args_):
        operands = list(args_)
        if partition_name is not None:
            operands.append(bass2jax.partition_id_tensor())
        outs = bass2jax._bass_exec_p.bind(
            *operands, out_avals=tuple(out_avals), in_names=tuple(bind_names),
            out_names=tuple(out_names), lowering_input_output_aliases=(),
            sim_require_finite=True, sim_require_nnan=True, nc=nc)
        return tuple(outs)

    mesh = Mesh(np.asarray(jax.devices()[:NCORES]), ("core",))
    n_outs = len(out_names)
    sm = shard_map(_body, mesh=mesh,
                   in_specs=(PartitionSpec("core"),) * (n_params + n_outs),
                   out_specs=(PartitionSpec("core"),) * n_outs,
                   check_rep=False)
    structs = (
        jax.ShapeDtypeStruct((B_ALL, T_FULL, D), np.int8),
        jax.ShapeDtypeStruct((NCORES * 128, PKC), ml_dtypes.bfloat16),
        jax.ShapeDtypeStruct((B_ALL, T_FULL, VOUT), np.uint8))
    # AOT compile with BassEffect suppressed -> C++ fast-path dispatch
    fn = bass2jax.fast_dispatch_compile(
        lambda: jax.jit(sm, keep_unused=True).lower(*structs).compile())
    sh = NamedSharding(mesh, PartitionSpec("core"))
    Z_dev = jax.device_put(
        np.zeros((NCORES * BL_WAVE, T_FULL, VOUT), np.uint8), sh)
    runner = {"fn": fn, "sh": sh, "Z": Z_dev, "in_names": in_names}
    _CACHE["runner"] = runner
    return runner


def _ensure_weights(runner, X, Wx_f, Wh_f, b_f, Wx_b, Wh_b, b_b, Wc, bc):
    """Device-resident packed weights keyed by content hash; returns (PK_dev, g0)."""
    wh = _hash_arrays([Wx_f, Wh_f, b_f, Wx_b, Wh_b, b_b, Wc, bc])
    st = _CACHE.get("weights")
    if st is not None and st["hash"] == wh:
        # g0 stays; quant-time extrema check handles X outgrowing it
        return st["PK_dev"], st["g0"]
    g0 = max(float(np.max(X)), -float(np.min(X)), 1e-30)
    pk = _pack_pk(Wx_f, Wh_f, b_f, Wx_b, Wh_b, b_b, Wc, bc, g0 / 127.0)
    pk_rep = np.ascontiguousarray(
        np.broadcast_to(pk[None], (NCORES, 128, PKC))).reshape(NCORES * 128, PKC)
    PK_dev = jax.device_put(pk_rep, runner["sh"])
    PK_dev.block_until_ready()
    _CACHE["weights"] = {"hash": wh, "g0": g0, "PK_dev": PK_dev}
    return PK_dev, g0


_QSTATE = {}


def _quant_put(X, scale, sh, pool):
    """Quantize per-core chunks and upload each to its device as it's ready.

    Thread c quantizes X[4c:4c+4] (1 MB int8) then device_puts it to core
    c's device alone, so later chunks' quantization overlaps earlier
    chunks' wire transfer; single-device puts issued from distinct threads
    pipeline at full link bandwidth. The 8 shards are assembled zero-copy
    into one global array for the compiled call.

    Returns (X_dev, fmax, fmin) with fmax/fmin extrema of X*scale.
    """
    if "q" not in _QSTATE:
        _QSTATE["q"] = [np.empty((BL_WAVE, T_FULL, D), np.int8)
                        for _ in range(NCORES)]
        _QSTATE["f"] = [np.empty((BL_WAVE, T_FULL, D), np.float32)
                        for _ in range(NCORES)]
        _QSTATE["devs"] = jax.devices()[:NCORES]
        # quant is memory-bound: cap concurrency so the first chunks finish
        # (and hit the wire) sooner instead of all 8 crawling in parallel
        _QSTATE["sem"] = threading.Semaphore(3)

    def work(c):
        f = _QSTATE["f"][c]
        q = _QSTATE["q"][c]
        with _QSTATE["sem"]:
            np.multiply(X[c * BL_WAVE:(c + 1) * BL_WAVE], scale, out=f)
            mx, mn = float(np.max(f)), float(np.min(f))
            np.rint(f, out=f)
            np.copyto(q, f, casting="unsafe")
        d = jax.device_put(q, _QSTATE["devs"][c])
        return d, mx, mn

    res = list(pool.map(work, range(NCORES)))
    X_dev = jax.make_array_from_single_device_arrays(
        (B_ALL, T_FULL, D), sh, [r[0] for r in res])
    fmax = max(r[1] for r in res)
    fmin = min(r[2] for r in res)
    return X_dev, fmax, fmin


def _decode_all(buf, out, pool):
    """buf [B,512,97] u8 -> out f32 [B,512,97]: p = q / sum(q) per row."""
    nb = buf.shape[0]
    nchunk = max(1, nb // N_QTHREADS)

    def work(lo):
        hi = min(lo + nchunk, nb)
        qv = buf[lo:hi].astype(np.float32)
        s = qv.sum(-1, keepdims=True)
        np.reciprocal(s, out=s)
        np.multiply(qv, s, out=out[lo:hi])

    if pool is None:
        for lo in range(0, nb, nchunk):
            work(lo)
    else:
        list(pool.map(work, range(0, nb, nchunk)))


def kernel(X, Wx_f, Wh_f, b_f, Wx_b, Wh_b, b_b, Wc, bc,
           label=None, inputlength=None, labellength=None):
    X = np.asarray(X, np.float32)
    with _LOCK:
        runner = _setup_runner()
        if "pool" not in _CACHE:
            _CACHE["pool"] = ThreadPoolExecutor(max_workers=NCORES)
        pool = _CACHE["pool"]
        sh = runner["sh"]
        st = _CACHE.get("weights")
        if st is not None:
            # warm path: start X quant+uploads with the cached calibration
            # immediately; the weight hash then runs while the wire drains
            g_used = st["g0"]
            X_dev, fmax, fmin = _quant_put(X, 127.0 / g_used, sh, pool)
            PK_dev, g0 = _ensure_weights(
                runner, X, Wx_f, Wh_f, b_f, Wx_b, Wh_b, b_b, Wc, bc)
            if g0 != g_used:  # weights changed -> recalibrated -> requant
                X_dev, fmax, fmin = _quant_put(X, 127.0 / g0, sh, pool)
        else:
            PK_dev, g0 = _ensure_weights(
                runner, X, Wx_f, Wh_f, b_f, Wx_b, Wh_b, b_b, Wc, bc)
            X_dev, fmax, fmin = _quant_put(X, 127.0 / g0, sh, pool)
        if fmax > 127.49 or fmin < -127.49:
            # X exceeds the cached calibration: re-fold weights with new g
            g0 = max(fmax, -fmin) * g0 / 127.0
            st = _CACHE["weights"]
            pk = _pack_pk(Wx_f, Wh_f, b_f, Wx_b, Wh_b, b_b, Wc, bc, g0 / 127.0)
            pk_rep = np.ascontiguousarray(
                np.broadcast_to(pk[None], (NCORES, 128, PKC))
            ).reshape(NCORES * 128, PKC)
            PK_dev = jax.device_put(pk_rep, sh)
            PK_dev.block_until_ready()
            st.update(g0=g0, PK_dev=PK_dev)
            X_dev, _, _ = _quant_put(X, 127.0 / g0, sh, pool)
        fn, Z = runner["fn"], runner["Z"]
        out = np.empty((B_ALL, T_FULL, V + 1), np.float32)
        outs = fn(X_dev, PK_dev, Z)
        # fetch per-core shards from threads (overlaps wire + decode)
        shards = outs[0].addressable_shards

        def fetch_dec(i):
            shd = shards[i]
            lo = shd.index[0].start or 0
            buf = np.asarray(shd.data)
            _decode_all(buf, out[lo:lo + buf.shape[0]], None)

        list(pool.map(fetch_dec, range(NCORES)))
    return out


if __name__ == "__main__":
    import reference
    ins = {k: np.asarray(v) for k, v in reference.setup_inputs().items()}
    got = kernel(**ins)
    want = np.asarray(reference.reference(**ins))
    err = np.abs(got - want).max() / np.abs(want).max()
    print("abs-rel err:", err)


# revision 42
# speedup vs baseline: 1.8908x; 1.0637x over previous
"""Trainium2 Bass kernel: cosine-attention + positional-adjacency mix + BiLSTM + softmax classifier.

Model (per sample, reference semantics):
    Xn   = X / ||X||_row
    Xa   = (Xn Xn^T) @ A_D @ X          (A_D = row-normalized exp(-|i-j|/8), constant)
    h    = BiLSTM(Xa)                    (fwd + bwd, H=256)
    out  = softmax(h @ Wc + bc)

Strategy: data-parallel over batch across 8 cores. All device matmuls in bf16
with fp32 PSUM accumulation; the feedforward runs in "transposed" layout so
the LSTM gate math operates on 128-partition tiles (see phase comments).

Wall-clock-per-call engineering (the metric is wall time of kernel(); the
axon tunnel to the devices has ~80 ms per-op latency and ~30-40 MB/s
aggregate bandwidth, so bytes moved on the wire dominate -- device compute
measures as negligible next to the transfers):
  - The shard_map executable is AOT-compiled ONCE (fast-dispatch, effects
    suppressed) and cached; repeat calls skip re-trace/re-lower/reload.
    This replaces per-call run_bass_kernel_spmd, which rebuilds the jit
    wrapper (and re-serializes the BIR) on every invocation.
  - Weights ship once (content-hashed): a single packed bf16 blob (PK),
    replicated to all 8 cores, stays device-resident across calls. The X
    int8 quantization scale is folded into Wx inside that blob, so X
    re-uploads never force weight re-uploads. No collectives in the
    program (a full weight copy sits on every core).
  - X ships as globally-scaled int8 (8.4 MB instead of 33.6 MB f32); the
    scale cancels in the cosine normalization and is folded into Wx for
    the value path. Sub-8-bit X was measured and rejected twice over:
    global int7/int6 quant alone costs 1.5e-2/2.9e-2 rel err vs the
    2e-2 gate, and a working 7-bit-packed per-row-scaled variant (rel
    err 8.7e-3, DVE shift/or unpack) measured ~60 ms SLOWER end-to-end
    in an interleaved A/B -- host packing passes plus odd 452 B row
    transfers cost more than the 1 MB wire saving.
  - Per-core upload pipeline: 8 worker threads each quantize their core's
    1 MB chunk then device_put it to that core alone (single-device puts
    from distinct threads pipeline at full link bandwidth; one big sharded
    put works too, but per-core puts overlap quantization with the wire).
    The shards are assembled zero-copy via
    make_array_from_single_device_arrays.
  - Output ships as uint8 (97 B/row instead of 97*2 B bf16): the device
    writes q = round(e * 254/max(e)); the host recovers the softmax as
    q/sum(q) -- rows sum to 1, so no scale bytes are needed. Fetch+decode
    run per-shard in threads.
  - The zero "donation" buffer for OUT is a device-resident dummy (the
    kernel writes every OUT element, so no per-call zeros upload and no
    donation -- the buffer is reused forever).
"""

import os
import threading
import zlib
from concurrent.futures import ThreadPoolExecutor

import numpy as np
import ml_dtypes

os.environ.setdefault("JAX_COMPILATION_CACHE_DIR", "/tmp/jaxcache")
import jax
try:
    jax.config.update("jax_compilation_cache_dir",
                      os.environ["JAX_COMPILATION_CACHE_DIR"])
    jax.config.update("jax_persistent_cache_min_entry_size_bytes", -1)
    jax.config.update("jax_persistent_cache_min_compile_time_secs", 0)
except Exception:
    pass

from jax.sharding import Mesh, PartitionSpec, NamedSharding
from jax.experimental.shard_map import shard_map

import concourse.bass as bass
from concourse.bass import ds
import concourse.mybir as mybir
import concourse.bacc as bacc
import concourse.tile as tile
from concourse import bass2jax

F32 = mybir.dt.float32
BF16 = mybir.dt.bfloat16
U8 = mybir.dt.uint8
I8 = mybir.dt.int8
AF = mybir.ActivationFunctionType
ALU = mybir.AluOpType
BF16NP = ml_dtypes.bfloat16

B_ALL, T_FULL, D, H, V = 32, 512, 512, 256, 96
SIGMA = 8.0
NCORES = 8
G4 = 4 * H                    # 1024 gate dims (permuted order i,f,o,g)
NM = G4 // 128                # 8 gate m-tiles
KD = D // 128                 # 4
KH = H // 128                 # 2
VP = 392                      # WC cols padded 4*(V+1)=388 -> 392
VOUT = V + 1                  # uint8 out row: 97 q values (host renormalizes)

# packed replicated weight blob (bf16 cols): WXf | WXb | WHf | WHb | WC | BIAS | BCREP
_WXC = KD * G4                # 4096
_WHC = KH * G4                # 2048
_C_WXF = 0
_C_WXB = _C_WXF + _WXC
_C_WHF = _C_WXB + _WXC
_C_WHB = _C_WHF + _WHC
_C_WC = _C_WHB + _WHC
_C_BIAS = _C_WC + VP          # f32 section (even offset)
_C_BC = _C_BIAS + 2 * (2 * NM)
PKC = _C_BC + 2 * (V + 1)     # 12906

BL_WAVE = 4                   # samples/core/execution (monolithic: one exec)
OUT_ROUND = float(os.environ.get("KROUND", "0.0"))  # f32->u8 rounding offset
N_QTHREADS = 4                # host-side quant/decode parallelism (= bl)


def _host_stationary(q):
    """[R, C] -> [128, (R//128)*C]; k-th col-block = rows k*128:(k+1)*128."""
    r, c = q.shape
    return np.ascontiguousarray(
        q.reshape(r // 128, 128, c).transpose(1, 0, 2).reshape(128, (r // 128) * c)
    )


def _permute_gates(w):
    """Reorder last-dim gate blocks [i,f,g,o] -> [i,f,o,g]."""
    i, f, g, o = np.split(w, 4, axis=-1)
    return np.concatenate([i, f, o, g], axis=-1)


def build_program(bl=BL_WAVE, n_devices=NCORES):
    T = T_FULL
    KT = T // 128
    nc = bacc.Bacc(
        "TRN2", target_bir_lowering=False, debug=False, enable_asserts=False,
        num_devices=n_devices,
    )

    x_in = nc.dram_tensor("XBF", [bl, T, D], I8, kind="ExternalInput")
    pk_in = nc.dram_tensor("PK", [128, PKC], BF16, kind="ExternalInput")
    out_d = nc.dram_tensor("OUT", [bl, T, VOUT], U8, kind="ExternalOutput")

    bias_in = pk_in[:, _C_BIAS:_C_BIAS + 4 * NM].bitcast(F32)
    bc_in = pk_in[:, _C_BC:PKC].bitcast(F32)

    B8 = 2 * bl    # h-cols per hstore slot
    GB = NM * bl   # z free cols

    with tile.TileContext(nc) as tc:
        with (
            tc.tile_pool(name="const", bufs=1) as cpool,
            tc.tile_pool(name="gates", bufs=1) as gpool,
            tc.tile_pool(name="state", bufs=1) as spool,
        ):
            wx_sb = {}
            wh_sb = {}
            for di, d in enumerate("fb"):
                wx_sb[d] = cpool.tile([128, _WXC], BF16, name=f"wx_{d}")
                nc.sync.dma_start(
                    wx_sb[d][:],
                    pk_in[:, (_C_WXF, _C_WXB)[di]:(_C_WXF, _C_WXB)[di] + _WXC])
                wh_sb[d] = cpool.tile([128, _WHC], BF16, name=f"wh_{d}")
                nc.sync.dma_start(
                    wh_sb[d][:],
                    pk_in[:, (_C_WHF, _C_WHB)[di]:(_C_WHF, _C_WHB)[di] + _WHC])
            wc_sb = cpool.tile([128, VP], BF16)
            nc.sync.dma_start(wc_sb[:], pk_in[:, _C_WC:_C_WC + VP])
            bias_sb = cpool.tile([128, 2 * NM], F32)
            nc.sync.dma_start(bias_sb[:], bias_in)
            bc_sb = cpool.tile([128, V + 1], F32)
            nc.sync.dma_start(bc_sb[:], bc_in)

            # A_D generated on device: ad_sb[p, k*T+c] = AD[k*128+p, c]
            ad_sb = cpool.tile([128, KT * T], BF16)
            with tc.tile_pool(name="adgen", bufs=2) as agp:
                for k in range(KT):
                    df = agp.tile([128, T], F32, tag="df")
                    nc.gpsimd.iota(df[:], pattern=[[1, T]], base=-(k * 128),
                                   channel_multiplier=-1,
                                   allow_small_or_imprecise_dtypes=True)
                    ab = agp.tile([128, T], F32, tag="ab")
                    nc.scalar.activation(ab[:], df[:], AF.Abs)
                    ex = agp.tile([128, T], F32, tag="ex")
                    ssum = agp.tile([128, 1], F32, tag="ssum")
                    nc.scalar.activation(ex[:], ab[:], AF.Exp,
                                         scale=-1.0 / SIGMA, accum_out=ssum[:])
                    rs = agp.tile([128, 1], F32, tag="rs")
                    nc.vector.reciprocal(rs[:], ssum[:])
                    nc.vector.tensor_scalar_mul(
                        ad_sb[:, k * T:(k + 1) * T], ex[:], rs[:])

            gates = {d: gpool.tile([128, NM * bl * T], BF16, name=f"gates_{d}")
                     for d in "fb"}
            hstore = {d: spool.tile([128, (T + 1) * B8], BF16, name=f"hstore_{d}")
                      for d in "fb"}
            cstate = {d: spool.tile([128, B8], F32, name=f"cstate_{d}") for d in "fb"}
            for d in "fb":
                z0 = 0 if d == "f" else T
                nc.vector.memset(hstore[d][:, z0 * B8:(z0 + 1) * B8], 0.0)
                nc.vector.memset(cstate[d][:], 0.0)

            # ---------------- Phase A: feedforward per sample ----------------
            with (
                tc.tile_pool(name="xb", bufs=2) as xbp,
                tc.tile_pool(name="mats", bufs=2) as mpool,
                tc.tile_pool(name="small", bufs=4) as smpool,
                tc.tile_pool(name="ps", bufs=4, space="PSUM") as pspool,
            ):
                with tc.For_i(0, bl, 1) as s:
                    x_bf = xbp.tile([128, KT, D], BF16, tag="x_bf")
                    x8 = xbp.tile([128, KT, D], I8, tag="x8")
                    nc.sync.dma_start(
                        x8[:].rearrange("p (o k) d -> p o k d", o=1),
                        x_in[ds(s, 1)].rearrange("o (k p) d -> p o k d", p=128))
                    nc.vector.tensor_copy(x_bf[:], x8[:])
                    xn_bf = xbp.tile([128, KT, D], BF16, tag="xn_bf")
                    xnt_bf = xbp.tile([128, KD, T], BF16, tag="xnt_bf")
                    dump = smpool.tile([128, D], BF16, tag="dump")
                    for k in range(KT):
                        ss = smpool.tile([128, 1], F32, tag="ss")
                        nc.scalar.activation(dump[:], x_bf[:, k, :], AF.Square,
                                             accum_out=ss[:])
                        sn = smpool.tile([128, 1], F32, tag="sn")
                        nc.scalar.activation(sn[:], ss[:], AF.Sqrt)
                        rn = smpool.tile([128, 1], F32, tag="rn")
                        nc.vector.reciprocal(rn[:], sn[:])
                        nc.vector.tensor_scalar_mul(xn_bf[:, k, :], x_bf[:, k, :], rn[:])
                    # Xn^T via DMA block transposes
                    for ti in range(KT):
                        for dj in range(KD):
                            nc.sync.dma_start_transpose(
                                xnt_bf[:, dj, ti * 128:(ti + 1) * 128],
                                xn_bf[:, ti, dj * 128:(dj + 1) * 128])
                    # A_S = Xn Xn^T   [T, T]
                    as_bf = mpool.tile([128, KT, T], BF16, tag="as_bf")
                    for m in range(KT):
                        ps = pspool.tile([128, T], F32, tag="psA")
                        for k in range(KD):
                            nc.tensor.matmul(
                                ps[:], xnt_bf[:, k, m * 128:(m + 1) * 128],
                                xnt_bf[:, k, :], start=(k == 0), stop=(k == KD - 1))
                        nc.vector.tensor_copy(as_bf[:, m, :], ps[:])
                    # P = A_D^T @ A_S
                    p_bf = mpool.tile([128, KT, T], BF16, tag="p_bf")
                    for m in range(KT):
                        ps = pspool.tile([128, T], F32, tag="psA")
                        for k in range(KT):
                            nc.tensor.matmul(
                                ps[:], ad_sb[:, k * T + m * 128:k * T + (m + 1) * 128],
                                as_bf[:, k, :], start=(k == 0), stop=(k == KT - 1))
                        nc.vector.tensor_copy(p_bf[:, m, :], ps[:])
                    # Xa^T = X^T @ P   [D, T]
                    xat_bf = mpool.tile([128, KD, T], BF16, tag="xat_bf")
                    for m in range(KD):
                        ps = pspool.tile([128, T], F32, tag="psA")
                        for k in range(KT):
                            nc.tensor.matmul(
                                ps[:], x_bf[:, k, m * 128:(m + 1) * 128],
                                p_bf[:, k, :], start=(k == 0), stop=(k == KT - 1))
                        nc.vector.tensor_copy(xat_bf[:, m, :], ps[:])
                    # gx^T = Wx^T @ Xa^T (+b) per direction
                    for di, d in enumerate("fb"):
                        for m in range(NM):
                            ps = pspool.tile([128, T], F32, tag="psA")
                            for k in range(KD):
                                nc.tensor.matmul(
                                    ps[:],
                                    wx_sb[d][:, k * G4 + m * 128:k * G4 + (m + 1) * 128],
                                    xat_bf[:, k, :], start=(k == 0), stop=(k == KD - 1))
                            nc.vector.tensor_scalar_add(
                                gates[d][:].rearrange(
                                    "p (t m s) -> p t m s",
                                    m=NM, s=bl)[:, :, m, ds(s, 1)],
                                ps[:].rearrange("p (t o) -> p t o", o=1),
                                bias_sb[:, di * NM + m:di * NM + m + 1])

            # ---------------- Phase R: BiLSTM recurrence (HW loop) ----------------
            with (
                tc.tile_pool(name="zps", bufs=4, space="PSUM") as zpool,
                tc.tile_pool(name="zsb", bufs=4) as zsbp,
                tc.tile_pool(name="sg", bufs=4) as sgp,
            ):
                with tc.For_i(0, T, 1) as i:
                    for d in "fb":
                        if d == "f":
                            roff = i * B8
                            woff = (i + 1) * B8
                            gcol = i * GB
                        else:
                            roff = (T - i) * B8
                            woff = (T - 1 - i) * B8
                            gcol = (T - 1 - i) * GB
                        hprev = sgp.tile([128, B8], BF16, tag=f"hprev_{d}")
                        nc.vector.tensor_copy(
                            hprev[:], hstore[d][:, ds(roff, B8)])
                        z_ps = zpool.tile([128, GB], F32, tag="z_ps")
                        for m in range(NM):
                            for j in range(KH):
                                nc.tensor.matmul(
                                    z_ps[:, m * bl:(m + 1) * bl],
                                    wh_sb[d][:, j * G4 + m * 128:j * G4 + (m + 1) * 128],
                                    hprev[:, j * bl:(j + 1) * bl],
                                    start=(j == 0), stop=(j == KH - 1))
                        z_sb = zsbp.tile([128, GB], F32, tag="z_sb")
                        nc.vector.scalar_tensor_tensor(
                            z_sb[:], z_ps[:], 1.0, gates[d][:, ds(gcol, GB)],
                            ALU.bypass, ALU.add)
                        sg = sgp.tile([128, GB], F32, tag="sg")
                        nc.scalar.activation(
                            sg[:, :6 * bl], z_sb[:, :6 * bl], AF.Sigmoid)
                        nc.scalar.activation(
                            sg[:, 6 * bl:], z_sb[:, 6 * bl:], AF.Tanh)
                        u = sgp.tile([128, B8], F32, tag="u")
                        nc.vector.scalar_tensor_tensor(
                            u[:], sg[:, :B8], 1.0, sg[:, 6 * bl:], ALU.bypass, ALU.mult)
                        q = sgp.tile([128, B8], F32, tag="q")
                        nc.vector.scalar_tensor_tensor(
                            q[:], sg[:, B8:2 * B8], 1.0, cstate[d][:],
                            ALU.bypass, ALU.mult)
                        nc.vector.scalar_tensor_tensor(
                            cstate[d][:], u[:], 1.0, q[:], ALU.bypass, ALU.add)
                        ct = sgp.tile([128, B8], F32, tag="ct")
                        nc.scalar.activation(ct[:], cstate[d][:], AF.Tanh)
                        nc.vector.scalar_tensor_tensor(
                            hstore[d][:, ds(woff, B8)],
                            sg[:, 2 * B8:3 * B8], 1.0, ct[:], ALU.bypass, ALU.mult)

            # ---------------- Phase C: classifier + softmax -> u8 ----------------
            with (
                tc.tile_pool(name="cps", bufs=4, space="PSUM") as cpsp,
                tc.tile_pool(name="csb", bufs=4) as csbp,
            ):
                NTB = T // 128
                out_flat = out_d[:].rearrange("s t v -> (s t) v")
                # [p, c, t] views: c = within-slot column (j*bl + sample),
                # t = slot index (stride B8)
                vw = {d: hstore[d][:].rearrange("p (t c) -> p c t", c=B8)
                      for d in "fb"}
                with tc.For_i(0, bl, 1) as s:
                    for m in range(NTB):
                        hst = csbp.tile([128, 4, 128], BF16, tag="hst")
                        for k in range(4):
                            # fwd h(t) lives at slot t+1, bwd h(t) at slot t
                            d = "f" if k < 2 else "b"
                            t0 = m * 128 + (1 if k < 2 else 0)
                            nc.vector.tensor_copy(
                                hst[:, k:k + 1, :],
                                vw[d][:, ds((k % 2) * bl + s, 1), t0:t0 + 128])
                        ps = cpsp.tile([128, V + 1], F32, tag="psC")
                        for k in range(4):
                            nc.tensor.matmul(
                                ps[:], hst[:, k, :],
                                wc_sb[:, k * (V + 1):(k + 1) * (V + 1)],
                                start=(k == 0), stop=(k == 3))
                        lg = csbp.tile([128, V + 1], F32, tag="lg")
                        nc.vector.scalar_tensor_tensor(
                            lg[:], ps[:], 1.0, bc_sb[:], ALU.bypass, ALU.add)
                        e = csbp.tile([128, V + 1], F32, tag="e")
                        esum = csbp.tile([128, 1], F32, tag="esum")
                        nc.scalar.activation(e[:], lg[:], AF.Exp,
                                             accum_out=esum[:])
                        # q = e*(254/max(e)) as u8; host renormalizes by sum(q)
                        # (softmax rows sum to 1, so no scale bytes needed)
                        rm = csbp.tile([128, 1], F32, tag="rm")
                        nc.vector.reduce_max(rm[:], e[:], axis=mybir.AxisListType.X)
                        rm254 = csbp.tile([128, 1], F32, tag="rm254")
                        nc.vector.tensor_scalar_mul(rm254[:], rm[:], 1.0 / 254.0)
                        rs254 = csbp.tile([128, 1], F32, tag="rs254")
                        nc.vector.reciprocal(rs254[:], rm254[:])
                        o = csbp.tile([128, VOUT], U8, tag="o")
                        nc.vector.tensor_scalar(
                            o[:, :V + 1], e[:], rs254[:], OUT_ROUND,
                            ALU.mult, ALU.add)
                        nc.sync.dma_start(
                            out_flat[ds(s * T + m * 128, 128), :], o[:])

    nc.compile()
    return nc


# ---------------------------------------------------------------------------
# Host-side runner: cached compiled executable + device-resident weights +
# threaded wave pipeline.
# ---------------------------------------------------------------------------

_CACHE = {}
_LOCK = threading.Lock()


def _hash_arrays(arrs):
    h = 0
    for a in arrs:
        a = np.ascontiguousarray(a)
        h = zlib.adler32(a.view(np.uint8).reshape(-1), h)
    return h


def _pack_pk(Wx_f, Wh_f, b_f, Wx_b, Wh_b, b_b, Wc, bc, wx_scale):
    """Full packed weight blob [128, PKC] bf16 (f32 sections bitcast)."""
    pk = np.empty((128, PKC), BF16NP)
    for di, (wx, wh) in enumerate([(Wx_f, Wh_f), (Wx_b, Wh_b)]):
        wxp = _permute_gates(np.asarray(wx, np.float32) * wx_scale)
        whp = _permute_gates(np.asarray(wh, np.float32))
        pk[:, (_C_WXF, _C_WXB)[di]:(_C_WXF, _C_WXB)[di] + _WXC] = \
            _host_stationary(wxp).astype(BF16NP)
        pk[:, (_C_WHF, _C_WHB)[di]:(_C_WHF, _C_WHB)[di] + _WHC] = \
            _host_stationary(whp).astype(BF16NP)
    wc_pack = np.zeros((128, VP), BF16NP)
    wc_pack[:, :4 * (V + 1)] = _host_stationary(
        np.asarray(Wc, np.float32)).astype(BF16NP)
    pk[:, _C_WC:_C_WC + VP] = wc_pack
    bias_cols = np.zeros((128, 2 * NM), np.float32)
    for di, b in enumerate([b_f, b_b]):
        bp = _permute_gates(np.asarray(b, np.float32))
        bias_cols[:, di * NM:(di + 1) * NM] = bp.reshape(NM, 128).T
    pk[:, _C_BIAS:_C_BIAS + 4 * NM] = bias_cols.view(BF16NP)
    bcrep = np.ascontiguousarray(
        np.broadcast_to(np.asarray(bc, np.float32), (128, V + 1)))
    pk[:, _C_BC:PKC] = bcrep.view(BF16NP)
    return pk


def _setup_runner():
    """Build program, jit it, warm the compile. Cached."""
    if "runner" in _CACHE:
        return _CACHE["runner"]
    nc = build_program(BL_WAVE)
    bass2jax.install_neuronx_cc_hook()

    partition_name = nc.partition_id_tensor.name if nc.partition_id_tensor else None
    in_names, out_names, out_avals = [], [], []
    for alloc in nc.m.functions[0].allocations:
        if not isinstance(alloc, mybir.MemoryLocationSet):
            continue
        name = alloc.memorylocations[0].name
        if alloc.kind == "ExternalInput":
            if name != partition_name:
                in_names.append(name)
        elif alloc.kind == "ExternalOutput":
            out_names.append(name)
            out_avals.append(jax.core.ShapedArray(
                tuple(alloc.tensor_shape), mybir.dt.np(alloc.dtype)))
    n_params = len(in_names)
    bind_names = in_names + out_names + ([partition_name] if partition_name else [])

    def _body(*args_):
        operands = list(args_)
        if partition_name is not None:
            operands.append(bass2jax.partition_id_tensor())
        outs = bass2jax._bass_exec_p.bind(
            *operands, out_avals=tuple(out_avals), in_names=tuple(bind_names),
            out_names=tuple(out_names), lowering_input_output_aliases=(),
            sim_require_finite=True, sim_require_nnan=True, nc=nc)
        return tuple(outs)

    mesh = Mesh(np.asarray(jax.devices()[:NCORES]), ("core",))
    n_outs = len(out_names)
    sm = shard_map(_body, mesh=mesh,
                   in_specs=(PartitionSpec("core"),) * (n_params + n_outs),
                   out_specs=(PartitionSpec("core"),) * n_outs,
                   check_rep=False)
    structs = (
        jax.ShapeDtypeStruct((B_ALL, T_FULL, D), np.int8),
        jax.ShapeDtypeStruct((NCORES * 128, PKC), ml_dtypes.bfloat16),
        jax.ShapeDtypeStruct((B_ALL, T_FULL, VOUT), np.uint8))
    # AOT compile with BassEffect suppressed -> C++ fast-path dispatch
    fn = bass2jax.fast_dispatch_compile(
        lambda: jax.jit(sm, keep_unused=True).lower(*structs).compile())
    sh = NamedSharding(mesh, PartitionSpec("core"))
    Z_dev = jax.device_put(
        np.zeros((NCORES * BL_WAVE, T_FULL, VOUT), np.uint8), sh)
    runner = {"fn": fn, "sh": sh, "Z": Z_dev, "in_names": in_names}
    _CACHE["runner"] = runner
    return runner


def _ensure_weights(runner, X, Wx_f, Wh_f, b_f, Wx_b, Wh_b, b_b, Wc, bc):
    """Device-resident packed weights keyed by content hash; returns (PK_dev, g0)."""
    wh = _hash_arrays([Wx_f, Wh_f, b_f, Wx_b, Wh_b, b_b, Wc, bc])
    st = _CACHE.get("weights")
    if st is not None and st["hash"] == wh:
        # g0 stays; quant-time extrema check handles X outgrowing it
        return st["PK_dev"], st["g0"]
    g0 = max(float(np.max(X)), -float(np.min(X)), 1e-30)
    pk = _pack_pk(Wx_f, Wh_f, b_f, Wx_b, Wh_b, b_b, Wc, bc, g0 / 127.0)
    pk_rep = np.ascontiguousarray(
        np.broadcast_to(pk[None], (NCORES, 128, PKC))).reshape(NCORES * 128, PKC)
    PK_dev = jax.device_put(pk_rep, runner["sh"])
    PK_dev.block_until_ready()
    _CACHE["weights"] = {"hash": wh, "g0": g0, "PK_dev": PK_dev}
    return PK_dev, g0


_QSTATE = {}


def _quant_put(X, scale, sh, pool):
    """Quantize per-core chunks and upload each to its device as it's ready.

    Thread c quantizes X[4c:4c+4] (1 MB int8) then device_puts it to core
    c's device alone, so later chunks' quantization overlaps earlier
    chunks' wire transfer; single-device puts issued from distinct threads
    pipeline at full link bandwidth. The 8 shards are assembled zero-copy
    into one global array for the compiled call.

    Returns (X_dev, fmax, fmin) with fmax/fmin extrema of X*scale.
    """
    if "q" not in _QSTATE:
        _QSTATE["q"] = [np.empty((BL_WAVE, T_FULL, D), np.int8)
                        for _ in range(NCORES)]
        _QSTATE["f"] = [np.empty((BL_WAVE, T_FULL, D), np.float32)
                        for _ in range(NCORES)]
        _QSTATE["devs"] = jax.devices()[:NCORES]
        # quant is memory-bound: cap concurrency so the first chunks finish
        # (and hit the wire) sooner instead of all 8 crawling in parallel
        _QSTATE["sem"] = threading.Semaphore(3)

    def work(c):
        f = _QSTATE["f"][c]
        q = _QSTATE["q"][c]
        with _QSTATE["sem"]:
            np.multiply(X[c * BL_WAVE:(c + 1) * BL_WAVE], scale, out=f)
            mx, mn = float(np.max(f)), float(np.min(f))
            np.rint(f, out=f)
            np.copyto(q, f, casting="unsafe")
        d = jax.device_put(q, _QSTATE["devs"][c])
        return d, mx, mn

    res = list(pool.map(work, range(NCORES)))
    X_dev = jax.make_array_from_single_device_arrays(
        (B_ALL, T_FULL, D), sh, [r[0] for r in res])
    fmax = max(r[1] for r in res)
    fmin = min(r[2] for r in res)
    return X_dev, fmax, fmin


def _decode_all(buf, out, pool):
    """buf [B,512,97] u8 -> out f32 [B,512,97]: p = q / sum(q) per row."""
    nb = buf.shape[0]
    nchunk = max(1, nb // N_QTHREADS)

    def work(lo):
        hi = min(lo + nchunk, nb)
        qv = buf[lo:hi].astype(np.float32)
        s = qv.sum(-1, keepdims=True)
        np.reciprocal(s, out=s)
        np.multiply(qv, s, out=out[lo:hi])

    if pool is None:
        for lo in range(0, nb, nchunk):
            work(lo)
    else:
        list(pool.map(work, range(0, nb, nchunk)))


def kernel(X, Wx_f, Wh_f, b_f, Wx_b, Wh_b, b_b, Wc, bc,
           label=None, inputlength=None, labellength=None):
    X = np.asarray(X, np.float32)
    with _LOCK:
        runner = _setup_runner()
        if "pool" not in _CACHE:
            _CACHE["pool"] = ThreadPoolExecutor(max_workers=NCORES)
        pool = _CACHE["pool"]
        sh = runner["sh"]
        st = _CACHE.get("weights")
        if st is not None:
            # warm path: start X quant+uploads with the cached calibration
            # immediately; the weight hash then runs while the wire drains
            g_used = st["g0"]
            X_dev, fmax, fmin = _quant_put(X, 127.0 / g_used, sh, pool)
            PK_dev, g0 = _ensure_weights(
                runner, X, Wx_f, Wh_f, b_f, Wx_b, Wh_b, b_b, Wc, bc)
            if g0 != g_used:  # weights changed -> recalibrated -> requant
                X_dev, fmax, fmin = _quant_put(X, 127.0 / g0, sh, pool)
        else:
            PK_dev, g0 = _ensure_weights(
                runner, X, Wx_f, Wh_f, b_f, Wx_b, Wh_b, b_b, Wc, bc)
            X_dev, fmax, fmin = _quant_put(X, 127.0 / g0, sh, pool)
        if fmax > 127.49 or fmin < -127.49:
            # X exceeds the cached calibration: re-fold weights with new g
            g0 = max(fmax, -fmin) * g0 / 127.0
            st = _CACHE["weights"]
            pk = _pack_pk(Wx_f, Wh_f, b_f, Wx_b, Wh_b, b_b, Wc, bc, g0 / 127.0)
            pk_rep = np.ascontiguousarray(
                np.broadcast_to(pk[None], (NCORES, 128, PKC))
            ).reshape(NCORES * 128, PKC)
            PK_dev = jax.device_put(pk_rep, sh)
            PK_dev.block_until_ready()
            st.update(g0=g0, PK_dev=PK_dev)
            X_dev, _, _ = _quant_put(X, 127.0 / g0, sh, pool)
        fn, Z = runner["fn"], runner["Z"]
        out = np.empty((B_ALL, T_FULL, V + 1), np.float32)
        outs = fn(X_dev, PK_dev, Z)
        # fetch per-core shards from threads (overlaps wire + decode)
        shards = outs[0].addressable_shards

        def fetch_dec(i):
            shd = shards[i]
            lo = shd.index[0].start or 0
            buf = np.asarray(shd.data)
            _decode_all(buf, out[lo:lo + buf.shape[0]], None)

        list(pool.map(fetch_dec, range(NCORES)))
    return out


if __name__ == "__main__":
    import reference
    ins = {k: np.asarray(v) for k, v in reference.setup_inputs().items()}
    got = kernel(**ins)
    want = np.asarray(reference.reference(**ins))
    err = np.abs(got - want).max() / np.abs(want).max()
    print("abs-rel err:", err)
